# revision 13
# baseline (speedup 1.0000x reference)
"""DiT attention block on 8 Trainium2 NeuronCores.

Reference computation (fp32):
    qkv = x @ Wqkv + b            (b=2, n=2048, din=1024, 3*1024)
    q, k = RMSNorm_full_dim(q|k) * scale  (norm over all 1024 channels)
    RoPE (rotary_dim=64) per 64-dim head, 16 heads
    attn = softmax(q k^T / 8) v ;  out = attn @ Wout + bout
    Sharding: 8 cores = 2 batches x 4 head-groups (4 heads / 256 features).

Two SPMD launches (fp16 data paths, fp32 accumulation):
  L1: qkv projection in transposed layout, kt-streamed so the PE computes
      behind the input DMA stream; RoPE applied on-chip (rotation commutes
      with the norm scale); partial sum-of-squares for the full-dim RMSNorm
      (weighted by 1/scale^2, so it must read the PRE-rope values); V packed
      [v_h|1] per head on-chip.  Host combines ssq -> rsqrt factors.
  L2: qhat = qR * broadcast(r_q) (host-materialized broadcast); khat = kR
      with r_k folded into the exp's per-partition scale operand.  Attention
      stream: S^T = kR^T qhat (PE, row-tiled head pairs), exp straight from
      PSUM (ACT, the bottleneck engine: ~1.09us per [128,1024] step), O^T
      = [V|1]^T P.  Normalization uses reciprocal_approx_fast + gpsimd
      partition_broadcast.  Out-projection partials are injected densely
      into the attention stream; host adds the partials + bias term.
"""

import os
import sys

for _p in ("/opt/trn_rl_repo", "/root/.axon_site/_ro/trn_rl_repo"):
    if os.path.isdir(_p) and _p not in sys.path:
        sys.path.append(_p)

import numpy as np

import concourse.bass as bass  # noqa: E402,F401
import concourse.mybir as mybir  # noqa: E402
import concourse.tile as tile  # noqa: E402
from concourse import bacc  # noqa: E402
from concourse.bass_utils import run_bass_kernel_spmd  # noqa: E402

FP32 = mybir.dt.float32
FP16 = mybir.dt.float16
AF = mybir.ActivationFunctionType

B = 2
N = 2048
DIN = 1024
DQ = 1024
H = 16
DH = 64
NCORES = 8
NGROUP = 4          # head-groups per batch
GF = 256            # features per core (4 heads)
P = 128
EPS = 1e-6
ROPE_BASE = 10000.0

LAST_EXEC_NS = {}   # filled when KERNEL_TRACE=1
LAST_RESULTS = {}   # BassKernelResults per launch when KERNEL_TRACE=1

_cache = {}


# ----------------------------------------------------------------- launch 1

def _build_l1():
    nc = bacc.Bacc("TRN2", target_bir_lowering=False, debug=False,
                   num_devices=NCORES)
    xT = nc.dram_tensor("xT", [DIN, N], FP16, kind="ExternalInput")
    wcat = nc.dram_tensor("wcat", [DIN, 3 * GF], FP16, kind="ExternalInput")
    bqk = nc.dram_tensor("bqk", [P, 4], FP32, kind="ExternalInput")
    invs = nc.dram_tensor("invs", [P, P], FP16, kind="ExternalInput")
    cosr = nc.dram_tensor("cosr", [P, N], FP16, kind="ExternalInput")
    sinr = nc.dram_tensor("sinr", [P, N], FP16, kind="ExternalInput")
    qR_o = nc.dram_tensor("qR", [GF, N], FP16, kind="ExternalOutput")
    kR_o = nc.dram_tensor("kR", [GF, N], FP16, kind="ExternalOutput")
    # v65 layout: token-major rows, per key-tile column groups of 4*(64+1)
    v_o = nc.dram_tensor("v65", [P, 16 * 260], FP16, kind="ExternalOutput")
    ssq_o = nc.dram_tensor("ssq", [2, N], FP32, kind="ExternalOutput")

    KT = DIN // P  # 8 contraction tiles

    with tile.TileContext(nc) as tc:
        with (
            tc.tile_pool(name="xw", bufs=1) as xw,
            tc.tile_pool(name="bigp", bufs=1) as bigp,
            tc.tile_pool(name="scr", bufs=2) as scr,
            tc.tile_pool(name="shp", bufs=2) as shp,
            tc.tile_pool(name="outq", bufs=2) as outq,
            tc.tile_pool(name="sqp", bufs=4) as sqp,
            tc.tile_pool(name="vst", bufs=1) as vst,
            tc.tile_pool(name="stgp", bufs=1) as stgp,
            tc.tile_pool(name="ps", bufs=4, space="PSUM") as ps,
        ):
            # ---- input DMAs: xt on sync queue, wt on scalar queue ----
            xt, wt = [], []
            for kt in range(KT):
                t = xw.tile([P, N], FP16, tag=f"xt{kt}")
                w = xw.tile([P, 3 * GF], FP16, tag=f"wt{kt}")
                if kt == 0:
                    nc.scalar.dma_start(w[:, 0:GF], wcat[0:P, 0:GF])
                    nc.sync.dma_start(t[:, 0:1024], xT[0:P, 0:1024])
                    nc.sync.dma_start(t[:, 1024:2048], xT[0:P, 1024:2048])
                    nc.scalar.dma_start(w[:, GF:3 * GF], wcat[0:P, GF:3 * GF])
                else:
                    nc.sync.dma_start(t[:], xT[kt * P:(kt + 1) * P, :])
                    nc.scalar.dma_start(w[:], wcat[kt * P:(kt + 1) * P, :])
                xt.append(t)
                wt.append(w)
            bias = xw.tile([P, 4], FP32, tag="bias")
            nc.scalar.dma_start(bias[:], bqk[:, :])
            winv = xw.tile([P, P], FP16, tag="winv")
            nc.scalar.dma_start(winv[:], invs[:, :])
            cosb = xw.tile([P, N], FP16, tag="cos")
            nc.scalar.dma_start(cosb[:], cosr[:, :])
            sinb = xw.tile([P, N], FP16, tag="sin")
            nc.scalar.dma_start(sinb[:], sinr[:, :])

            # v staging: 16 key-tiles x (4 heads x 65); ones columns set once
            vstage = vst.tile([P, 16 * 260], FP16, tag="vstage")
            ones = vst.tile([P, 64], FP16, tag="ones")
            nc.vector.memset(ones[:], 1.0)
            nc.vector.tensor_copy(
                vstage[:].rearrange("p (a h c) -> p a h c", a=16, h=4)[
                    :, :, :, 64:65],
                ones[:].rearrange("p (a h c) -> p a h c", a=16, h=4))

            sq = {}

            def qk_phase(t_idx, out_dram, streamed):
                # projection for one of q/k in transposed layout.
                col0 = t_idx * GF
                accs = [ps.tile([P, 1024], FP32, tag="acc",
                                name=f"acc{t_idx}_{i}") for i in range(4)]
                if streamed:
                    # kt-outer: compute behind the input DMA stream
                    for kt in range(KT):
                        for mt in range(2):
                            for nb in range(4):
                                nc.tensor.matmul(
                                    accs[mt * 2 + nb // 2][
                                        :, (nb % 2) * 512:(nb % 2 + 1) * 512],
                                    wt[kt][:, col0 + mt * P:col0 + (mt + 1) * P],
                                    xt[kt][:, nb * 512:(nb + 1) * 512],
                                    start=(kt == 0), stop=(kt == KT - 1),
                                )
                else:
                    for mt in range(2):
                        for nb in range(4):
                            for kt in range(KT):
                                nc.tensor.matmul(
                                    accs[mt * 2 + nb // 2][
                                        :, (nb % 2) * 512:(nb % 2 + 1) * 512],
                                    wt[kt][:, col0 + mt * P:col0 + (mt + 1) * P],
                                    xt[kt][:, nb * 512:(nb + 1) * 512],
                                    start=(kt == 0), stop=(kt == KT - 1),
                                )
                dmae = nc.scalar if t_idx == 0 else nc.sync
                for mt in range(2):
                    big = bigp.tile([P, N], FP16, tag=f"big{t_idx}_{mt}")
                    for nbp in range(2):
                        nc.scalar.activation(
                            big[:, nbp * 1024:(nbp + 1) * 1024],
                            accs[mt * 2 + nbp][:], AF.Identity,
                            bias=bias[:, 2 * t_idx + mt:2 * t_idx + mt + 1])
                    # pre-rope squares for the weighted ssq (gpsimd: DVE
                    # is saturated by the rope chain)
                    s = sqp.tile([P, N], FP16, tag=f"sq{t_idx}_{mt}")
                    nc.gpsimd.tensor_mul(s[:], big[:], big[:])
                    sq[(t_idx, mt)] = s
                    # rope: rotate_half via 4 partition-block DMAs
                    sh = shp.tile([P, N], FP16, tag="sh")
                    for blk in range(4):
                        srcb = blk ^ 1
                        (dmae if blk < 2 else nc.gpsimd).dma_start(
                            sh[blk * 32:(blk + 1) * 32, :],
                            big[srcb * 32:(srcb + 1) * 32, :])
                    t2 = scr.tile([P, N], FP16, tag="t2")
                    nc.vector.tensor_mul(t2[:], big[:], cosb[:])
                    nc.vector.tensor_mul(sh[:], sh[:], sinb[:])
                    rr = outq.tile([P, N], FP16, tag="rr")
                    nc.vector.tensor_add(rr[:], t2[:], sh[:])
                    dmae.dma_start(out_dram[mt * P:(mt + 1) * P, :], rr[:])

            qk_phase(0, qR_o, streamed=True)
            qk_phase(1, kR_o, streamed=False)

            # ---- ssq: 32 identical output rows via all-equal lhsT columns ----
            stg = stgp.tile([1, 2 * N], FP32, tag="stg")
            for t_idx in range(2):
                for np2 in range(2):
                    sp = ps.tile([32, 1024], FP32, tag="acc",
                                 name=f"ssq{t_idx}_{np2}")
                    for nbi in range(2):
                        nb = np2 * 2 + nbi
                        for mt in range(2):
                            nc.tensor.matmul(
                                sp[:, nbi * 512:(nbi + 1) * 512],
                                winv[:, 32 * (2 * t_idx + mt):
                                     32 * (2 * t_idx + mt + 1)],
                                sq[(t_idx, mt)][:, nb * 512:(nb + 1) * 512],
                                start=(mt == 0), stop=(mt == 1),
                            )
                    nc.scalar.copy(
                        stg[0:1, t_idx * N + np2 * 1024:
                            t_idx * N + (np2 + 1) * 1024],
                        sp[0:1, :])
            for t_idx in range(2):
                nc.sync.dma_start(ssq_o[t_idx:t_idx + 1, :],
                                  stg[0:1, t_idx * N:(t_idx + 1) * N])

            # ---- v phase (tiles resident now) ----
            for tp in range(4):
                vacc = ps.tile([P, 1024], FP32, tag="acc", name=f"vacc{tp}")
                for ti in range(4):
                    tt = tp * 4 + ti
                    for kt in range(KT):
                        nc.tensor.matmul(
                            vacc[:, ti * 256:(ti + 1) * 256],
                            xt[kt][:, tt * P:(tt + 1) * P],
                            wt[kt][:, 2 * GF:3 * GF],
                            start=(kt == 0), stop=(kt == KT - 1),
                        )
                nc.vector.tensor_copy(
                    vstage[:].rearrange("p (a h c) -> p a h c", a=16, h=4)[
                        :, tp * 4:(tp + 1) * 4, :, 0:64],
                    vacc[:].rearrange("p (a h c) -> p a h c", a=4, h=4))
            nc.scalar.dma_start(v_o[:, :], vstage[:])

    nc.compile()
    return nc


# ----------------------------------------------------------------- launch 2

def _build_l2():
    nc = bacc.Bacc("TRN2", target_bir_lowering=False, debug=False,
                   num_devices=NCORES)
    qR = nc.dram_tensor("qR", [GF, N], FP16, kind="ExternalInput")
    kR = nc.dram_tensor("kR", [GF, N], FP16, kind="ExternalInput")
    rqb_i = nc.dram_tensor("rqb", [P, N], FP16, kind="ExternalInput")
    rkb_i = nc.dram_tensor("rkb", [P, N], FP16, kind="ExternalInput")
    v_i = nc.dram_tensor("v65", [P, 16 * 260], FP16, kind="ExternalInput")
    wout_i = nc.dram_tensor("wout", [GF, DIN], FP16, kind="ExternalInput")
    part_o = nc.dram_tensor("part", [2, N, DIN], FP16, kind="ExternalOutput")

    IBW = 512        # query-block width
    NIB = N // IBW   # 4 query blocks
    NJT = N // P     # 16 key tiles

    with tile.TileContext(nc) as tc:
        with (
            tc.tile_pool(name="cst", bufs=1) as cst,
            tc.tile_pool(name="hatp", bufs=1) as hatp,
            tc.tile_pool(name="ptp", bufs=4) as ptp,
            tc.tile_pool(name="obig", bufs=1) as obigp,
            tc.tile_pool(name="onrm", bufs=2) as onrm,
            tc.tile_pool(name="outp", bufs=2) as outp,
            tc.tile_pool(name="tiny", bufs=4) as tiny,
            tc.tile_pool(name="psS", bufs=2, space="PSUM") as psS,
            tc.tile_pool(name="psA", bufs=1, space="PSUM") as psA,
            tc.tile_pool(name="psO", bufs=2, space="PSUM") as psO,
        ):
            # ---- loads: pair-0 tensors first so attention starts early ----
            kRt, qRt = [], []
            for mt in range(2):
                kt_ = cst.tile([P, N], FP16, tag=f"kR{mt}")
                qt_ = cst.tile([P, N], FP16, tag=f"qR{mt}")
                kRt.append(kt_)
                qRt.append(qt_)
            rqb = cst.tile([P, N], FP16, tag="rqb")
            rkb = cst.tile([P, N], FP16, tag="rkb")
            nc.sync.dma_start(kRt[0][:], kR[0:P, :])
            nc.scalar.dma_start(rkb[:], rkb_i[:, :])
            nc.sync.dma_start(qRt[0][:], qR[0:P, :])
            nc.sync.dma_start(rqb[:], rqb_i[:, :])
            vbig = cst.tile([P, 16 * 260], FP16, tag="v")
            nc.sync.dma_start(vbig[:], v_i[:, :])
            nc.sync.dma_start(kRt[1][:], kR[P:2 * P, :])
            nc.sync.dma_start(qRt[1][:], qR[P:2 * P, :])
            wout = []
            for kt in range(2):
                w = cst.tile([P, DIN], FP16, tag=f"wo{kt}")
                nc.sync.dma_start(w[:], wout_i[kt * P:(kt + 1) * P, :])
                wout.append(w)

            qhat, khat = [], []
            for mt in range(2):
                hk = hatp.tile([P, N], FP16, tag=f"khat{mt}")
                nc.vector.tensor_mul(hk[:], kRt[mt][:], rkb[:])
                khat.append(hk)
                h = hatp.tile([P, N], FP16, tag=f"qhat{mt}")
                nc.vector.tensor_mul(h[:], qRt[mt][:], rqb[:])
                qhat.append(h)

            def vt_slice(jt, h):
                return vbig[:, jt * 260 + h * 65:jt * 260 + (h + 1) * 65]

            # ---- attention ----
            obig = [obigp.tile([P, N], FP16, tag=f"obig{pr}", name=f"ob{pr}")
                    for pr in range(2)]

            osb_pend = {}

            def emit_proj(pr, tt, eng):
                pss = [psO.tile([P, 512], FP32, tag="O",
                                name=f"pj{pr}_{tt}_{hf}")
                       for hf in range(2)]
                for half in range(2):
                    nc.tensor.matmul(
                        pss[half][:],
                        obig[pr][:, tt * P:(tt + 1) * P],
                        wout[pr][:, half * 512:(half + 1) * 512],
                        start=True, stop=True,
                    )
                if tt % 2 == 0:
                    osb_pend[pr] = outp.tile([P, 2 * DIN], FP16, tag="osb",
                                             name=f"osb{pr}_{tt}")
                osb = osb_pend[pr]
                base = (tt % 2) * DIN
                eng.tensor_copy(osb[:, base:base + 512], pss[0][:])
                eng.tensor_copy(osb[:, base + 512:base + 1024], pss[1][:])
                if tt % 2 == 1:
                    nc.sync.dma_start(
                        part_o[pr, (tt - 1) * P:(tt + 1) * P, :].rearrange(
                            "(a p) d -> p a d", p=P),
                        osb[:].rearrange("p (a d) -> p a d", a=2))

            def finish_block(pr, ib, o_ps):
                rinv, bc = [], []
                for sub in range(2):
                    d = tiny.tile([1, IBW], FP32, tag="dsb",
                                  name=f"d{pr}_{ib}_{sub}")
                    nc.vector.tensor_copy(d[:, :], o_ps[sub][64:65, :])
                    t = tiny.tile([1, IBW], FP32, tag="rinv",
                                  name=f"ri{pr}_{ib}_{sub}")
                    nc.vector.reciprocal_approx_fast(t[:, :], d[:, :])
                    rinv.append(t)
                for sub in range(2):
                    t = tiny.tile([64, IBW], FP32, tag="bc",
                                  name=f"bc{pr}_{ib}_{sub}")
                    nc.gpsimd.partition_broadcast(t[:, :], rinv[sub][:, :])
                    bc.append(t)
                nc.vector.tensor_mul(
                    obig[pr][0:64, ib * IBW:(ib + 1) * IBW],
                    o_ps[0][0:64, :], bc[0][:, :])
                onr = onrm.tile([64, IBW], FP16, tag="onr")
                nc.vector.tensor_mul(onr[:, :], o_ps[1][0:64, :], bc[1][:, :])
                nc.sync.dma_start(
                    obig[pr][64:128, ib * IBW:(ib + 1) * IBW], onr[:, :])

            steps = [(pr, ib, jt) for pr in range(2) for ib in range(NIB)
                     for jt in range(NJT)]
            p_sbs = {}
            o_ps_map = {}
            ready = []      # projection token-tiles whose obig cols are done
            launched = []

            def emit_s(step):
                pr, ib, jt = step
                s_ps = psS.tile([P, 2 * IBW], FP32, tag="S")
                for sub in range(2):
                    nc.tensor.matmul(
                        s_ps[:, sub * IBW:(sub + 1) * IBW],
                        khat[pr][sub * 64:(sub + 1) * 64, jt * P:(jt + 1) * P],
                        qhat[pr][sub * 64:(sub + 1) * 64,
                                 ib * IBW:(ib + 1) * IBW],
                        start=True, stop=True,
                        tile_position=(64 * sub, 0),
                    )
                p_sb = ptp.tile([P, 2 * IBW], FP16, tag="P",
                                name=f"p{pr}_{ib}_{jt}")
                nc.scalar.activation(p_sb[:, :], s_ps[:, :],
                                     AF.Exp, scale=0.125)
                p_sbs[step] = p_sb

            nproj = 0
            emit_s(steps[0])
            emit_s(steps[1])
            for si, step in enumerate(steps):
                pr, ib, jt = step
                if si + 2 < len(steps):
                    emit_s(steps[si + 2])
                if (pr, ib) not in o_ps_map:
                    o_ps_map[(pr, ib)] = [
                        psA.tile([65, IBW], FP32, tag=f"oacc{s}",
                                 name=f"o{pr}_{ib}_{s}") for s in range(2)]
                o_ps = o_ps_map[(pr, ib)]
                p_sb = p_sbs.pop(step)
                for sub in range(2):
                    h = 2 * pr + sub
                    nc.tensor.matmul(
                        o_ps[sub][:, :],
                        vt_slice(jt, h),
                        p_sb[:, sub * IBW:(sub + 1) * IBW],
                        start=(jt == 0), stop=(jt == NJT - 1),
                    )
                if jt in (1, 4, 7, 10) and ready:
                    ptt = ready.pop(0)
                    emit_proj(ptt[0], ptt[1], nc.vector)
                    nproj += 1
                if jt == NJT - 1:
                    finish_block(pr, ib, o_ps)
                    ready.extend((pr, ib * 4 + i) for i in range(4))

            for ptt in ready:
                emit_proj(ptt[0], ptt[1], nc.vector)
                nproj += 1

    nc.compile()
    return nc


# ------------------------------------------------------------------- driver

def _rope_tables():
    half = DH // 2
    inv_freq = 1.0 / (ROPE_BASE ** (np.arange(half, dtype=np.float64) * 2.0
                                    / DH))
    freqs = np.arange(N, dtype=np.float64)[:, None] * inv_freq[None, :]
    cos = np.cos(freqs).T          # (32, N)
    sin = np.sin(freqs).T
    cos64 = np.concatenate([cos, cos], 0)            # (64, N)
    sin64 = np.concatenate([-sin, sin], 0)           # signed for rotate_half
    cos_t = np.ascontiguousarray(
        np.concatenate([cos64, cos64], 0).astype(np.float16))  # (128, N)
    sin_t = np.ascontiguousarray(
        np.concatenate([sin64, sin64], 0).astype(np.float16))
    return cos_t, sin_t


def kernel(input, w_qkv, b_qkv, q_scale, k_scale, w_out, b_out):
    trace = bool(os.environ.get("KERNEL_TRACE"))
    if "l1" not in _cache:
        _cache["l1"] = _build_l1()
    if "l2" not in _cache:
        _cache["l2"] = _build_l2()

    x = np.asarray(input, dtype=np.float32)
    w_qkv = np.asarray(w_qkv, dtype=np.float32)
    b_qkv = np.asarray(b_qkv, dtype=np.float32)
    qs = np.asarray(q_scale, dtype=np.float32)
    ks = np.asarray(k_scale, dtype=np.float32)
    w_out = np.asarray(w_out, dtype=np.float32)
    b_out = np.asarray(b_out, dtype=np.float32)

    wq = w_qkv[:, :DQ] * qs[None, :]
    wk = w_qkv[:, DQ:2 * DQ] * ks[None, :]
    wv = w_qkv[:, 2 * DQ:]
    bq = b_qkv[:DQ] * qs
    bk = b_qkv[DQ:2 * DQ] * ks
    bv = b_qkv[2 * DQ:]

    xT = [np.ascontiguousarray(x[b].T.astype(np.float16)) for b in range(B)]
    cos_t, sin_t = _rope_tables()

    def col4(vec256_a, vec256_b):
        # -> (128, 4): [a_mt0 | a_mt1 | b_mt0 | b_mt1]
        return np.ascontiguousarray(np.stack(
            [vec256_a[:P], vec256_a[P:], vec256_b[:P], vec256_b[P:]],
            axis=1).astype(np.float32))

    in1 = []
    for c in range(NCORES):
        b, g = divmod(c, NGROUP)
        sl = slice(g * GF, (g + 1) * GF)
        wcat = np.ascontiguousarray(np.concatenate(
            [wq[:, sl], wk[:, sl], wv[:, sl]], axis=1).astype(np.float16))
        in1.append({
            "xT": xT[b],
            "wcat": wcat,
            "bqk": col4(bq[sl], bk[sl]),
            "invs": np.ascontiguousarray(np.repeat(
                col4(1.0 / np.square(qs[sl]), 1.0 / np.square(ks[sl])),
                32, axis=1).astype(np.float16)),
            "cosr": cos_t,
            "sinr": sin_t,
        })

    r1 = run_bass_kernel_spmd(_cache["l1"], in1,
                              core_ids=list(range(NCORES)), trace=trace)
    if trace:
        LAST_EXEC_NS["l1"] = r1.exec_time_ns
        LAST_RESULTS["l1"] = r1

    # host: combine partial ssq -> rsqrt factors
    tabs = {}
    for b in range(B):
        sq_q = np.zeros(N, np.float64)
        sq_k = np.zeros(N, np.float64)
        for g in range(NGROUP):
            ssq = r1.results[NGROUP * b + g]["ssq"].astype(np.float64)
            sq_q += ssq[0]
            sq_k += ssq[1]
        r_q = (1.0 / np.sqrt(sq_q / DQ + EPS)).astype(np.float32)
        r_k = (1.0 / np.sqrt(sq_k / DQ + EPS)).astype(np.float32)
        tabs[b] = {
            "rqb": np.ascontiguousarray(np.broadcast_to(
                r_q[None, :].astype(np.float16), (P, N))),
            "rkb": np.ascontiguousarray(np.broadcast_to(
                r_k[None, :].astype(np.float16), (P, N))),
        }

    in2 = []
    for c in range(NCORES):
        b, g = divmod(c, NGROUP)
        sl = slice(g * GF, (g + 1) * GF)
        in2.append({
            "qR": r1.results[c]["qR"],
            "kR": r1.results[c]["kR"],
            "v65": r1.results[c]["v65"],
            "wout": np.ascontiguousarray(w_out[sl, :].astype(np.float16)),
            **tabs[b],
        })

    r2 = run_bass_kernel_spmd(_cache["l2"], in2,
                              core_ids=list(range(NCORES)), trace=trace)
    if trace:
        LAST_EXEC_NS["l2"] = r2.exec_time_ns
        LAST_RESULTS["l2"] = r2

    base = (bv.astype(np.float64) @ w_out.astype(np.float64)
            + b_out.astype(np.float64))
    out = np.zeros((B, N, DIN), np.float32)
    for b in range(B):
        acc = np.zeros((N, DIN), np.float64)
        for g in range(NGROUP):
            p = r2.results[NGROUP * b + g]["part"].astype(np.float64)
            acc += p[0]
            acc += p[1]
        out[b] = (acc + base[None, :]).astype(np.float32)
    return out


# revision 15
# speedup vs baseline: 1.0565x; 1.0565x over previous
"""DiT attention block on 8 Trainium2 NeuronCores.

Reference computation (fp32):
    qkv = x @ Wqkv + b            (b=2, n=2048, din=1024, 3*1024)
    q, k = RMSNorm_full_dim(q|k) * scale  (norm over all 1024 channels)
    RoPE (rotary_dim=64) per 64-dim head, 16 heads
    attn = softmax(q k^T / 8) v ;  out = attn @ Wout + bout
    Sharding: 8 cores = 2 batches x 4 head-groups (4 heads / 256 features).

Two SPMD launches (fp16 data paths, fp32 accumulation):
  L1: qkv projection in transposed layout, kt-streamed so the PE computes
      behind the input DMA stream; RoPE applied on-chip (rotation commutes
      with the norm scale); partial sum-of-squares for the full-dim RMSNorm
      (weighted by 1/scale^2, so it must read the PRE-rope values); V packed
      [v_h|1] per head on-chip.  Host combines ssq -> rsqrt factors.
  L2: qhat = qR * broadcast(r_q) (host-materialized broadcast); khat = kR
      with r_k folded into the exp's per-partition scale operand.  Attention
      stream: S^T = kR^T qhat (PE, row-tiled head pairs), exp straight from
      PSUM (ACT, the bottleneck engine: ~1.09us per [128,1024] step), O^T
      = [V|1]^T P.  Normalization uses reciprocal_approx_fast + gpsimd
      partition_broadcast.  Out-projection partials are injected densely
      into the attention stream; host adds the partials + bias term.
"""

import os
import sys

for _p in ("/opt/trn_rl_repo", "/root/.axon_site/_ro/trn_rl_repo"):
    if os.path.isdir(_p) and _p not in sys.path:
        sys.path.append(_p)

import numpy as np

import concourse.bass as bass  # noqa: E402,F401
import concourse.mybir as mybir  # noqa: E402
import concourse.tile as tile  # noqa: E402
from concourse import bacc  # noqa: E402
from concourse.bass_utils import run_bass_kernel_spmd  # noqa: E402

FP32 = mybir.dt.float32
FP16 = mybir.dt.float16
AF = mybir.ActivationFunctionType

B = 2
N = 2048
DIN = 1024
DQ = 1024
H = 16
DH = 64
NCORES = 8
NGROUP = 4          # head-groups per batch
GF = 256            # features per core (4 heads)
P = 128
EPS = 1e-6
ROPE_BASE = 10000.0

LAST_EXEC_NS = {}   # filled when KERNEL_TRACE=1
LAST_RESULTS = {}   # BassKernelResults per launch when KERNEL_TRACE=1

_cache = {}


# ----------------------------------------------------------------- launch 1

def _build_l1():
    nc = bacc.Bacc("TRN2", target_bir_lowering=False, debug=False,
                   num_devices=NCORES)
    xT = nc.dram_tensor("xT", [DIN, N], FP16, kind="ExternalInput")
    wcat = nc.dram_tensor("wcat", [DIN, 3 * GF], FP16, kind="ExternalInput")
    bqk = nc.dram_tensor("bqk", [P, 4], FP32, kind="ExternalInput")
    invs = nc.dram_tensor("invs", [P, P], FP16, kind="ExternalInput")
    cosr = nc.dram_tensor("cosr", [P, N], FP16, kind="ExternalInput")
    sinr = nc.dram_tensor("sinr", [P, N], FP16, kind="ExternalInput")
    qR_o = nc.dram_tensor("qR", [GF, N], FP16, kind="ExternalOutput")
    kR_o = nc.dram_tensor("kR", [GF, N], FP16, kind="ExternalOutput")
    # v65 layout: token-major rows, per key-tile column groups of 4*(64+1)
    v_o = nc.dram_tensor("v65", [P, 16 * 260], FP16, kind="ExternalOutput")
    ssq_o = nc.dram_tensor("ssq", [2, N], FP32, kind="ExternalOutput")

    KT = DIN // P  # 8 contraction tiles

    with tile.TileContext(nc) as tc:
        with (
            tc.tile_pool(name="xw", bufs=1) as xw,
            tc.tile_pool(name="bigp", bufs=1) as bigp,
            tc.tile_pool(name="scr", bufs=2) as scr,
            tc.tile_pool(name="shp", bufs=2) as shp,
            tc.tile_pool(name="outq", bufs=2) as outq,
            tc.tile_pool(name="sqp", bufs=4) as sqp,
            tc.tile_pool(name="vst", bufs=1) as vst,
            tc.tile_pool(name="stgp", bufs=1) as stgp,
            tc.tile_pool(name="ps", bufs=4, space="PSUM") as ps,
        ):
            # ---- input DMAs: xt on sync queue, wt on scalar queue ----
            xt, wt = [], []
            for kt in range(KT):
                t = xw.tile([P, N], FP16, tag=f"xt{kt}")
                w = xw.tile([P, 3 * GF], FP16, tag=f"wt{kt}")
                if kt == 0:
                    nc.scalar.dma_start(w[:, 0:GF], wcat[0:P, 0:GF])
                    nc.sync.dma_start(t[:, 0:1024], xT[0:P, 0:1024])
                    nc.sync.dma_start(t[:, 1024:2048], xT[0:P, 1024:2048])
                    nc.scalar.dma_start(w[:, GF:3 * GF], wcat[0:P, GF:3 * GF])
                else:
                    nc.sync.dma_start(t[:], xT[kt * P:(kt + 1) * P, :])
                    nc.scalar.dma_start(w[:], wcat[kt * P:(kt + 1) * P, :])
                xt.append(t)
                wt.append(w)
            bias = xw.tile([P, 4], FP32, tag="bias")
            nc.scalar.dma_start(bias[:], bqk[:, :])
            winv = xw.tile([P, P], FP16, tag="winv")
            nc.scalar.dma_start(winv[:], invs[:, :])
            cosb = xw.tile([P, N], FP16, tag="cos")
            nc.scalar.dma_start(cosb[:], cosr[:, :])
            sinb = xw.tile([P, N], FP16, tag="sin")
            nc.scalar.dma_start(sinb[:], sinr[:, :])

            # v staging: 16 key-tiles x (4 heads x 65); ones columns set once
            vstage = vst.tile([P, 16 * 260], FP16, tag="vstage")
            ones = vst.tile([P, 64], FP16, tag="ones")
            nc.vector.memset(ones[:], 1.0)
            nc.vector.tensor_copy(
                vstage[:].rearrange("p (a h c) -> p a h c", a=16, h=4)[
                    :, :, :, 64:65],
                ones[:].rearrange("p (a h c) -> p a h c", a=16, h=4))

            sq = {}

            def qk_phase(t_idx, out_dram, streamed):
                # projection for one of q/k in transposed layout.
                col0 = t_idx * GF
                accs = [ps.tile([P, 1024], FP32, tag="acc",
                                name=f"acc{t_idx}_{i}") for i in range(4)]
                if streamed:
                    # kt-outer: compute behind the input DMA stream
                    for kt in range(KT):
                        for mt in range(2):
                            for nb in range(4):
                                nc.tensor.matmul(
                                    accs[mt * 2 + nb // 2][
                                        :, (nb % 2) * 512:(nb % 2 + 1) * 512],
                                    wt[kt][:, col0 + mt * P:col0 + (mt + 1) * P],
                                    xt[kt][:, nb * 512:(nb + 1) * 512],
                                    start=(kt == 0), stop=(kt == KT - 1),
                                )
                else:
                    for mt in range(2):
                        for nb in range(4):
                            for kt in range(KT):
                                nc.tensor.matmul(
                                    accs[mt * 2 + nb // 2][
                                        :, (nb % 2) * 512:(nb % 2 + 1) * 512],
                                    wt[kt][:, col0 + mt * P:col0 + (mt + 1) * P],
                                    xt[kt][:, nb * 512:(nb + 1) * 512],
                                    start=(kt == 0), stop=(kt == KT - 1),
                                )
                dmae = nc.scalar if t_idx == 0 else nc.sync
                for mt in range(2):
                    big = bigp.tile([P, N], FP16, tag=f"big{t_idx}_{mt}")
                    s = sqp.tile([P, N], FP16, tag=f"sq{t_idx}_{mt}")
                    for nbp in range(2):
                        nc.scalar.activation(
                            big[:, nbp * 1024:(nbp + 1) * 1024],
                            accs[mt * 2 + nbp][:], AF.Identity,
                            bias=bias[:, 2 * t_idx + mt:2 * t_idx + mt + 1])
                        # pre-rope squares for the weighted ssq, fused as
                        # (acc + bias)^2 on the ACT engine (idle here)
                        nc.scalar.activation(
                            s[:, nbp * 1024:(nbp + 1) * 1024],
                            accs[mt * 2 + nbp][:], AF.Square,
                            bias=bias[:, 2 * t_idx + mt:2 * t_idx + mt + 1])
                    sq[(t_idx, mt)] = s
                    # rope: rotate_half via 4 partition-block DMAs
                    sh = shp.tile([P, N], FP16, tag="sh")
                    for blk in range(4):
                        srcb = blk ^ 1
                        (dmae if blk < 2 else nc.gpsimd).dma_start(
                            sh[blk * 32:(blk + 1) * 32, :],
                            big[srcb * 32:(srcb + 1) * 32, :])
                    t2 = scr.tile([P, N], FP16, tag="t2")
                    nc.vector.tensor_mul(t2[:], big[:], cosb[:])
                    nc.vector.tensor_mul(sh[:], sh[:], sinb[:])
                    rr = outq.tile([P, N], FP16, tag="rr")
                    nc.vector.tensor_add(rr[:], t2[:], sh[:])
                    dmae.dma_start(out_dram[mt * P:(mt + 1) * P, :], rr[:])

            qk_phase(0, qR_o, streamed=True)
            qk_phase(1, kR_o, streamed=False)

            # ---- ssq: 32 identical output rows via all-equal lhsT columns ----
            stg = stgp.tile([1, 2 * N], FP32, tag="stg")
            for t_idx in range(2):
                for np2 in range(2):
                    sp = ps.tile([32, 1024], FP32, tag="acc",
                                 name=f"ssq{t_idx}_{np2}")
                    for nbi in range(2):
                        nb = np2 * 2 + nbi
                        for mt in range(2):
                            nc.tensor.matmul(
                                sp[:, nbi * 512:(nbi + 1) * 512],
                                winv[:, 32 * (2 * t_idx + mt):
                                     32 * (2 * t_idx + mt + 1)],
                                sq[(t_idx, mt)][:, nb * 512:(nb + 1) * 512],
                                start=(mt == 0), stop=(mt == 1),
                            )
                    nc.scalar.copy(
                        stg[0:1, t_idx * N + np2 * 1024:
                            t_idx * N + (np2 + 1) * 1024],
                        sp[0:1, :])
            for t_idx in range(2):
                nc.sync.dma_start(ssq_o[t_idx:t_idx + 1, :],
                                  stg[0:1, t_idx * N:(t_idx + 1) * N])

            # ---- v phase (tiles resident now) ----
            for tp in range(4):
                vacc = ps.tile([P, 1024], FP32, tag="acc", name=f"vacc{tp}")
                for ti in range(4):
                    tt = tp * 4 + ti
                    for kt in range(KT):
                        nc.tensor.matmul(
                            vacc[:, ti * 256:(ti + 1) * 256],
                            xt[kt][:, tt * P:(tt + 1) * P],
                            wt[kt][:, 2 * GF:3 * GF],
                            start=(kt == 0), stop=(kt == KT - 1),
                        )
                nc.scalar.copy(
                    vstage[:].rearrange("p (a h c) -> p a h c", a=16, h=4)[
                        :, tp * 4:(tp + 1) * 4, :, 0:64],
                    vacc[:].rearrange("p (a h c) -> p a h c", a=4, h=4))
            nc.scalar.dma_start(v_o[:, :], vstage[:])

    nc.compile()
    return nc


# ----------------------------------------------------------------- launch 2

def _build_l2():
    nc = bacc.Bacc("TRN2", target_bir_lowering=False, debug=False,
                   num_devices=NCORES)
    qR = nc.dram_tensor("qR", [GF, N], FP16, kind="ExternalInput")
    kR = nc.dram_tensor("kR", [GF, N], FP16, kind="ExternalInput")
    rqb_i = nc.dram_tensor("rqb", [P, N], FP16, kind="ExternalInput")
    rkb_i = nc.dram_tensor("rkb", [P, N], FP16, kind="ExternalInput")
    v_i = nc.dram_tensor("v65", [P, 16 * 260], FP16, kind="ExternalInput")
    wout_i = nc.dram_tensor("wout", [GF, DIN], FP16, kind="ExternalInput")
    part_o = nc.dram_tensor("part", [2, N, DIN], FP16, kind="ExternalOutput")

    IBW = 512        # query-block width
    NIB = N // IBW   # 4 query blocks
    NJT = N // P     # 16 key tiles

    with tile.TileContext(nc) as tc:
        with (
            tc.tile_pool(name="cst", bufs=1) as cst,
            tc.tile_pool(name="hatp", bufs=1) as hatp,
            tc.tile_pool(name="ptp", bufs=6) as ptp,
            tc.tile_pool(name="obig", bufs=1) as obigp,
            tc.tile_pool(name="onrm", bufs=2) as onrm,
            tc.tile_pool(name="outp", bufs=2) as outp,
            tc.tile_pool(name="tiny", bufs=4) as tiny,
            tc.tile_pool(name="psS", bufs=2, space="PSUM") as psS,
            tc.tile_pool(name="psA", bufs=1, space="PSUM") as psA,
            tc.tile_pool(name="psO", bufs=2, space="PSUM") as psO,
        ):
            # ---- loads: pair-0 tensors first so attention starts early ----
            kRt, qRt = [], []
            for mt in range(2):
                kt_ = cst.tile([P, N], FP16, tag=f"kR{mt}")
                qt_ = cst.tile([P, N], FP16, tag=f"qR{mt}")
                kRt.append(kt_)
                qRt.append(qt_)
            rqb = cst.tile([P, N], FP16, tag="rqb")
            rkb = cst.tile([P, N], FP16, tag="rkb")
            nc.sync.dma_start(kRt[0][:], kR[0:P, :])
            nc.scalar.dma_start(rkb[:], rkb_i[:, :])
            nc.sync.dma_start(qRt[0][:], qR[0:P, :])
            nc.scalar.dma_start(rqb[:], rqb_i[:, :])
            vbig = cst.tile([P, 16 * 260], FP16, tag="v")
            nc.sync.dma_start(vbig[:], v_i[:, :])
            nc.sync.dma_start(kRt[1][:], kR[P:2 * P, :])
            nc.sync.dma_start(qRt[1][:], qR[P:2 * P, :])
            wout = []
            for kt in range(2):
                w = cst.tile([P, DIN], FP16, tag=f"wo{kt}")
                nc.sync.dma_start(w[:], wout_i[kt * P:(kt + 1) * P, :])
                wout.append(w)

            qhat, khat = [], []
            for mt in range(2):
                hk = hatp.tile([P, N], FP16, tag=f"khat{mt}")
                nc.vector.tensor_mul(hk[:], kRt[mt][:], rkb[:])
                khat.append(hk)
                h = hatp.tile([P, N], FP16, tag=f"qhat{mt}")
                nc.vector.tensor_mul(h[:], qRt[mt][:], rqb[:])
                qhat.append(h)

            def vt_slice(jt, h):
                return vbig[:, jt * 260 + h * 65:jt * 260 + (h + 1) * 65]

            # ---- attention ----
            obig = [obigp.tile([P, N], FP16, tag=f"obig{pr}", name=f"ob{pr}")
                    for pr in range(2)]

            osb_pend = {}

            def emit_proj(pr, tt, eng, use_act=False):
                pss = [psO.tile([P, 512], FP32, tag="O",
                                name=f"pj{pr}_{tt}_{hf}")
                       for hf in range(2)]
                for half in range(2):
                    nc.tensor.matmul(
                        pss[half][:],
                        obig[pr][:, tt * P:(tt + 1) * P],
                        wout[pr][:, half * 512:(half + 1) * 512],
                        start=True, stop=True,
                    )
                if tt % 2 == 0:
                    osb_pend[pr] = outp.tile([P, 2 * DIN], FP16, tag="osb",
                                             name=f"osb{pr}_{tt}")
                osb = osb_pend[pr]
                base = (tt % 2) * DIN
                if use_act:
                    nc.scalar.copy(osb[:, base:base + 512], pss[0][:])
                    nc.scalar.copy(osb[:, base + 512:base + 1024], pss[1][:])
                else:
                    eng.tensor_copy(osb[:, base:base + 512], pss[0][:])
                    eng.tensor_copy(osb[:, base + 512:base + 1024], pss[1][:])
                if tt % 2 == 1:
                    nc.sync.dma_start(
                        part_o[pr, (tt - 1) * P:(tt + 1) * P, :].rearrange(
                            "(a p) d -> p a d", p=P),
                        osb[:].rearrange("p (a d) -> p a d", a=2))

            def finish_block(pr, ib, o_ps):
                rinv, bc = [], []
                for sub in range(2):
                    d = tiny.tile([1, IBW], FP32, tag="dsb",
                                  name=f"d{pr}_{ib}_{sub}")
                    nc.vector.tensor_copy(d[:, :], o_ps[sub][64:65, :])
                    t = tiny.tile([1, IBW], FP32, tag="rinv",
                                  name=f"ri{pr}_{ib}_{sub}")
                    nc.vector.reciprocal_approx_fast(t[:, :], d[:, :])
                    rinv.append(t)
                for sub in range(2):
                    t = tiny.tile([64, IBW], FP32, tag="bc",
                                  name=f"bc{pr}_{ib}_{sub}")
                    nc.gpsimd.partition_broadcast(t[:, :], rinv[sub][:, :])
                    bc.append(t)
                nc.vector.tensor_mul(
                    obig[pr][0:64, ib * IBW:(ib + 1) * IBW],
                    o_ps[0][0:64, :], bc[0][:, :])
                onr = onrm.tile([64, IBW], FP16, tag="onr")
                nc.vector.tensor_mul(onr[:, :], o_ps[1][0:64, :], bc[1][:, :])
                nc.sync.dma_start(
                    obig[pr][64:128, ib * IBW:(ib + 1) * IBW], onr[:, :])

            steps = [(pr, ib, jt) for pr in range(2) for ib in range(NIB)
                     for jt in range(NJT)]
            p_sbs = {}
            o_ps_map = {}
            ready = []      # projection token-tiles whose obig cols are done
            launched = []

            def emit_s(step):
                pr, ib, jt = step
                s_ps = psS.tile([P, 2 * IBW], FP32, tag="S")
                for sub in range(2):
                    nc.tensor.matmul(
                        s_ps[:, sub * IBW:(sub + 1) * IBW],
                        khat[pr][sub * 64:(sub + 1) * 64, jt * P:(jt + 1) * P],
                        qhat[pr][sub * 64:(sub + 1) * 64,
                                 ib * IBW:(ib + 1) * IBW],
                        start=True, stop=True,
                        tile_position=(64 * sub, 0),
                    )
                p_sb = ptp.tile([P, 2 * IBW], FP16, tag="P",
                                name=f"p{pr}_{ib}_{jt}")
                nc.scalar.activation(p_sb[:, :], s_ps[:, :],
                                     AF.Exp, scale=0.125)
                p_sbs[step] = p_sb

            nproj = 0
            for k in range(4):
                emit_s(steps[k])
            for si, step in enumerate(steps):
                pr, ib, jt = step
                if si + 4 < len(steps):
                    emit_s(steps[si + 4])
                if (pr, ib) not in o_ps_map:
                    o_ps_map[(pr, ib)] = [
                        psA.tile([65, IBW], FP32, tag=f"oacc{s}",
                                 name=f"o{pr}_{ib}_{s}") for s in range(2)]
                o_ps = o_ps_map[(pr, ib)]
                p_sb = p_sbs.pop(step)
                for sub in range(2):
                    h = 2 * pr + sub
                    nc.tensor.matmul(
                        o_ps[sub][:, :],
                        vt_slice(jt, h),
                        p_sb[:, sub * IBW:(sub + 1) * IBW],
                        start=(jt == 0), stop=(jt == NJT - 1),
                    )
                if jt in (1, 4, 7, 10) and ready:
                    ptt = ready.pop(0)
                    emit_proj(ptt[0], ptt[1], nc.vector)
                    nproj += 1
                if jt == NJT - 1:
                    finish_block(pr, ib, o_ps)
                    ready.extend((pr, ib * 4 + i) for i in range(4))

            for ptt in ready:
                emit_proj(ptt[0], ptt[1], nc.vector, use_act=True)
                nproj += 1

    nc.compile()
    return nc


# ------------------------------------------------------------------- driver

def _rope_tables():
    half = DH // 2
    inv_freq = 1.0 / (ROPE_BASE ** (np.arange(half, dtype=np.float64) * 2.0
                                    / DH))
    freqs = np.arange(N, dtype=np.float64)[:, None] * inv_freq[None, :]
    cos = np.cos(freqs).T          # (32, N)
    sin = np.sin(freqs).T
    cos64 = np.concatenate([cos, cos], 0)            # (64, N)
    sin64 = np.concatenate([-sin, sin], 0)           # signed for rotate_half
    cos_t = np.ascontiguousarray(
        np.concatenate([cos64, cos64], 0).astype(np.float16))  # (128, N)
    sin_t = np.ascontiguousarray(
        np.concatenate([sin64, sin64], 0).astype(np.float16))
    return cos_t, sin_t


def kernel(input, w_qkv, b_qkv, q_scale, k_scale, w_out, b_out):
    trace = bool(os.environ.get("KERNEL_TRACE"))
    if "l1" not in _cache:
        _cache["l1"] = _build_l1()
    if "l2" not in _cache:
        _cache["l2"] = _build_l2()

    x = np.asarray(input, dtype=np.float32)
    w_qkv = np.asarray(w_qkv, dtype=np.float32)
    b_qkv = np.asarray(b_qkv, dtype=np.float32)
    qs = np.asarray(q_scale, dtype=np.float32)
    ks = np.asarray(k_scale, dtype=np.float32)
    w_out = np.asarray(w_out, dtype=np.float32)
    b_out = np.asarray(b_out, dtype=np.float32)

    wq = w_qkv[:, :DQ] * qs[None, :]
    wk = w_qkv[:, DQ:2 * DQ] * ks[None, :]
    wv = w_qkv[:, 2 * DQ:]
    bq = b_qkv[:DQ] * qs
    bk = b_qkv[DQ:2 * DQ] * ks
    bv = b_qkv[2 * DQ:]

    xT = [np.ascontiguousarray(x[b].T.astype(np.float16)) for b in range(B)]
    cos_t, sin_t = _rope_tables()

    def col4(vec256_a, vec256_b):
        # -> (128, 4): [a_mt0 | a_mt1 | b_mt0 | b_mt1]
        return np.ascontiguousarray(np.stack(
            [vec256_a[:P], vec256_a[P:], vec256_b[:P], vec256_b[P:]],
            axis=1).astype(np.float32))

    in1 = []
    for c in range(NCORES):
        b, g = divmod(c, NGROUP)
        sl = slice(g * GF, (g + 1) * GF)
        wcat = np.ascontiguousarray(np.concatenate(
            [wq[:, sl], wk[:, sl], wv[:, sl]], axis=1).astype(np.float16))
        in1.append({
            "xT": xT[b],
            "wcat": wcat,
            "bqk": col4(bq[sl], bk[sl]),
            "invs": np.ascontiguousarray(np.repeat(
                col4(1.0 / np.square(qs[sl]), 1.0 / np.square(ks[sl])),
                32, axis=1).astype(np.float16)),
            "cosr": cos_t,
            "sinr": sin_t,
        })

    r1 = run_bass_kernel_spmd(_cache["l1"], in1,
                              core_ids=list(range(NCORES)), trace=trace)
    if trace:
        LAST_EXEC_NS["l1"] = r1.exec_time_ns
        LAST_RESULTS["l1"] = r1

    # host: combine partial ssq -> rsqrt factors
    tabs = {}
    for b in range(B):
        sq_q = np.zeros(N, np.float64)
        sq_k = np.zeros(N, np.float64)
        for g in range(NGROUP):
            ssq = r1.results[NGROUP * b + g]["ssq"].astype(np.float64)
            sq_q += ssq[0]
            sq_k += ssq[1]
        r_q = (1.0 / np.sqrt(sq_q / DQ + EPS)).astype(np.float32)
        r_k = (1.0 / np.sqrt(sq_k / DQ + EPS)).astype(np.float32)
        tabs[b] = {
            "rqb": np.ascontiguousarray(np.broadcast_to(
                r_q[None, :].astype(np.float16), (P, N))),
            "rkb": np.ascontiguousarray(np.broadcast_to(
                r_k[None, :].astype(np.float16), (P, N))),
        }

    in2 = []
    for c in range(NCORES):
        b, g = divmod(c, NGROUP)
        sl = slice(g * GF, (g + 1) * GF)
        in2.append({
            "qR": r1.results[c]["qR"],
            "kR": r1.results[c]["kR"],
            "v65": r1.results[c]["v65"],
            "wout": np.ascontiguousarray(w_out[sl, :].astype(np.float16)),
            **tabs[b],
        })

    r2 = run_bass_kernel_spmd(_cache["l2"], in2,
                              core_ids=list(range(NCORES)), trace=trace)
    if trace:
        LAST_EXEC_NS["l2"] = r2.exec_time_ns
        LAST_RESULTS["l2"] = r2

    base = (bv.astype(np.float64) @ w_out.astype(np.float64)
            + b_out.astype(np.float64))
    out = np.zeros((B, N, DIN), np.float32)
    for b in range(B):
        acc = np.zeros((N, DIN), np.float64)
        for g in range(NGROUP):
            p = r2.results[NGROUP * b + g]["part"].astype(np.float64)
            acc += p[0]
            acc += p[1]
        out[b] = (acc + base[None, :]).astype(np.float32)
    return out


# revision 16
# speedup vs baseline: 1.0646x; 1.0077x over previous
"""DiT attention block on 8 Trainium2 NeuronCores.

Reference computation (fp32):
    qkv = x @ Wqkv + b            (b=2, n=2048, din=1024, 3*1024)
    q, k = RMSNorm_full_dim(q|k) * scale  (norm over all 1024 channels)
    RoPE (rotary_dim=64) per 64-dim head, 16 heads
    attn = softmax(q k^T / 8) v ;  out = attn @ Wout + bout
    Sharding: 8 cores = 2 batches x 4 head-groups (4 heads / 256 features).

Two SPMD launches (fp16 data paths, fp32 accumulation):
  L1: qkv projection in transposed layout, kt-streamed so the PE computes
      behind the input DMA stream; RoPE applied on-chip (rotation commutes
      with the norm scale); partial sum-of-squares for the full-dim RMSNorm
      (weighted by 1/scale^2, so it must read the PRE-rope values); V packed
      [v_h|1] per head on-chip.  Host combines ssq -> rsqrt factors.
  L2: qhat = qR * broadcast(r_q) (host-materialized broadcast); khat = kR
      with r_k folded into the exp's per-partition scale operand.  Attention
      stream: S^T = kR^T qhat (PE, row-tiled head pairs), exp straight from
      PSUM (ACT, the bottleneck engine: ~1.09us per [128,1024] step), O^T
      = [V|1]^T P.  Normalization uses reciprocal_approx_fast + gpsimd
      partition_broadcast.  Out-projection partials are injected densely
      into the attention stream; host adds the partials + bias term.
"""

import os
import sys

for _p in ("/opt/trn_rl_repo", "/root/.axon_site/_ro/trn_rl_repo"):
    if os.path.isdir(_p) and _p not in sys.path:
        sys.path.append(_p)

import numpy as np

import concourse.bass as bass  # noqa: E402,F401
import concourse.mybir as mybir  # noqa: E402
import concourse.tile as tile  # noqa: E402
from concourse import bacc  # noqa: E402
from concourse.bass_utils import run_bass_kernel_spmd  # noqa: E402

FP32 = mybir.dt.float32
FP16 = mybir.dt.float16
AF = mybir.ActivationFunctionType

B = 2
N = 2048
DIN = 1024
DQ = 1024
H = 16
DH = 64
NCORES = 8
NGROUP = 4          # head-groups per batch
GF = 256            # features per core (4 heads)
P = 128
EPS = 1e-6
ROPE_BASE = 10000.0

LAST_EXEC_NS = {}   # filled when KERNEL_TRACE=1
LAST_RESULTS = {}   # BassKernelResults per launch when KERNEL_TRACE=1

_cache = {}


# ----------------------------------------------------------------- launch 1

def _build_l1():
    nc = bacc.Bacc("TRN2", target_bir_lowering=False, debug=False,
                   num_devices=NCORES)
    xT = nc.dram_tensor("xT", [DIN, N], FP16, kind="ExternalInput")
    wcat = nc.dram_tensor("wcat", [DIN, 3 * GF], FP16, kind="ExternalInput")
    bqk = nc.dram_tensor("bqk", [P, 4], FP32, kind="ExternalInput")
    invs = nc.dram_tensor("invs", [P, P], FP16, kind="ExternalInput")
    cosr = nc.dram_tensor("cosr", [P, N], FP16, kind="ExternalInput")
    sinr = nc.dram_tensor("sinr", [P, N], FP16, kind="ExternalInput")
    qR_o = nc.dram_tensor("qR", [GF, N], FP16, kind="ExternalOutput")
    kR_o = nc.dram_tensor("kR", [GF, N], FP16, kind="ExternalOutput")
    # v65 layout: token-major rows, per key-tile column groups of 4*(64+1)
    v_o = nc.dram_tensor("v65", [P, 16 * 260], FP16, kind="ExternalOutput")
    ssq_o = nc.dram_tensor("ssq", [2, N], FP32, kind="ExternalOutput")

    KT = DIN // P  # 8 contraction tiles

    with tile.TileContext(nc) as tc:
        with (
            tc.tile_pool(name="xw", bufs=1) as xw,
            tc.tile_pool(name="bigp", bufs=1) as bigp,
            tc.tile_pool(name="scr", bufs=2) as scr,
            tc.tile_pool(name="shp", bufs=2) as shp,
            tc.tile_pool(name="outq", bufs=2) as outq,
            tc.tile_pool(name="sqp", bufs=4) as sqp,
            tc.tile_pool(name="vst", bufs=1) as vst,
            tc.tile_pool(name="stgp", bufs=1) as stgp,
            tc.tile_pool(name="ps", bufs=4, space="PSUM") as ps,
        ):
            # ---- input DMAs: xt on sync queue, wt on scalar queue ----
            xt, wt = [], []
            for kt in range(KT):
                t = xw.tile([P, N], FP16, tag=f"xt{kt}")
                w = xw.tile([P, 3 * GF], FP16, tag=f"wt{kt}")
                if kt == 0:
                    nc.scalar.dma_start(w[:, 0:GF], wcat[0:P, 0:GF])
                    nc.sync.dma_start(t[:, 0:1024], xT[0:P, 0:1024])
                    nc.sync.dma_start(t[:, 1024:2048], xT[0:P, 1024:2048])
                    nc.scalar.dma_start(w[:, GF:3 * GF], wcat[0:P, GF:3 * GF])
                else:
                    nc.sync.dma_start(t[:], xT[kt * P:(kt + 1) * P, :])
                    nc.scalar.dma_start(w[:], wcat[kt * P:(kt + 1) * P, :])
                xt.append(t)
                wt.append(w)
            bias = xw.tile([P, 4], FP32, tag="bias")
            nc.scalar.dma_start(bias[:], bqk[:, :])
            winv = xw.tile([P, P], FP16, tag="winv")
            nc.scalar.dma_start(winv[:], invs[:, :])
            cosb = xw.tile([P, N], FP16, tag="cos")
            nc.scalar.dma_start(cosb[:], cosr[:, :])
            sinb = xw.tile([P, N], FP16, tag="sin")
            nc.scalar.dma_start(sinb[:], sinr[:, :])

            # v staging: 16 key-tiles x (4 heads x 65); ones columns set once
            vstage = vst.tile([P, 16 * 260], FP16, tag="vstage")
            ones = vst.tile([P, 64], FP16, tag="ones")
            nc.vector.memset(ones[:], 1.0)
            nc.vector.tensor_copy(
                vstage[:].rearrange("p (a h c) -> p a h c", a=16, h=4)[
                    :, :, :, 64:65],
                ones[:].rearrange("p (a h c) -> p a h c", a=16, h=4))

            sq = {}

            def qk_phase(t_idx, out_dram, streamed):
                # projection for one of q/k in transposed layout.
                col0 = t_idx * GF
                accs = [ps.tile([P, 1024], FP32, tag="acc",
                                name=f"acc{t_idx}_{i}") for i in range(4)]
                if streamed:
                    # kt-outer: compute behind the input DMA stream
                    for kt in range(KT):
                        for mt in range(2):
                            for nb in range(4):
                                nc.tensor.matmul(
                                    accs[mt * 2 + nb // 2][
                                        :, (nb % 2) * 512:(nb % 2 + 1) * 512],
                                    wt[kt][:, col0 + mt * P:col0 + (mt + 1) * P],
                                    xt[kt][:, nb * 512:(nb + 1) * 512],
                                    start=(kt == 0), stop=(kt == KT - 1),
                                )
                else:
                    for mt in range(2):
                        for nb in range(4):
                            for kt in range(KT):
                                nc.tensor.matmul(
                                    accs[mt * 2 + nb // 2][
                                        :, (nb % 2) * 512:(nb % 2 + 1) * 512],
                                    wt[kt][:, col0 + mt * P:col0 + (mt + 1) * P],
                                    xt[kt][:, nb * 512:(nb + 1) * 512],
                                    start=(kt == 0), stop=(kt == KT - 1),
                                )
                dmae = nc.scalar if t_idx == 0 else nc.sync
                for mt in range(2):
                    big = bigp.tile([P, N], FP16, tag=f"big{t_idx}_{mt}")
                    s = sqp.tile([P, N], FP16, tag=f"sq{t_idx}_{mt}")
                    for nbp in range(2):
                        nc.scalar.activation(
                            big[:, nbp * 1024:(nbp + 1) * 1024],
                            accs[mt * 2 + nbp][:], AF.Identity,
                            bias=bias[:, 2 * t_idx + mt:2 * t_idx + mt + 1])
                        # pre-rope squares for the weighted ssq, fused as
                        # (acc + bias)^2 on the ACT engine (idle here)
                        nc.scalar.activation(
                            s[:, nbp * 1024:(nbp + 1) * 1024],
                            accs[mt * 2 + nbp][:], AF.Square,
                            bias=bias[:, 2 * t_idx + mt:2 * t_idx + mt + 1])
                    sq[(t_idx, mt)] = s
                    # rope: rotate_half via 4 partition-block DMAs
                    sh = shp.tile([P, N], FP16, tag="sh")
                    for blk in range(4):
                        srcb = blk ^ 1
                        (dmae if blk < 2 else nc.gpsimd).dma_start(
                            sh[blk * 32:(blk + 1) * 32, :],
                            big[srcb * 32:(srcb + 1) * 32, :])
                    t2 = scr.tile([P, N], FP16, tag="t2")
                    nc.vector.tensor_mul(t2[:], big[:], cosb[:])
                    nc.vector.tensor_mul(sh[:], sh[:], sinb[:])
                    rr = outq.tile([P, N], FP16, tag="rr")
                    nc.vector.tensor_add(rr[:], t2[:], sh[:])
                    dmae.dma_start(out_dram[mt * P:(mt + 1) * P, :], rr[:])

            qk_phase(0, qR_o, streamed=True)
            qk_phase(1, kR_o, streamed=False)

            # ---- ssq: 32 identical output rows via all-equal lhsT columns ----
            stg = stgp.tile([1, 2 * N], FP32, tag="stg")
            for t_idx in range(2):
                for np2 in range(2):
                    sp = ps.tile([32, 1024], FP32, tag="acc",
                                 name=f"ssq{t_idx}_{np2}")
                    for nbi in range(2):
                        nb = np2 * 2 + nbi
                        for mt in range(2):
                            nc.tensor.matmul(
                                sp[:, nbi * 512:(nbi + 1) * 512],
                                winv[:, 32 * (2 * t_idx + mt):
                                     32 * (2 * t_idx + mt + 1)],
                                sq[(t_idx, mt)][:, nb * 512:(nb + 1) * 512],
                                start=(mt == 0), stop=(mt == 1),
                            )
                    nc.scalar.copy(
                        stg[0:1, t_idx * N + np2 * 1024:
                            t_idx * N + (np2 + 1) * 1024],
                        sp[0:1, :])
            for t_idx in range(2):
                nc.sync.dma_start(ssq_o[t_idx:t_idx + 1, :],
                                  stg[0:1, t_idx * N:(t_idx + 1) * N])

            # ---- v phase (tiles resident now) ----
            for tp in range(4):
                vacc = ps.tile([P, 1024], FP32, tag="acc", name=f"vacc{tp}")
                for ti in range(4):
                    tt = tp * 4 + ti
                    for kt in range(KT):
                        nc.tensor.matmul(
                            vacc[:, ti * 256:(ti + 1) * 256],
                            xt[kt][:, tt * P:(tt + 1) * P],
                            wt[kt][:, 2 * GF:3 * GF],
                            start=(kt == 0), stop=(kt == KT - 1),
                        )
                nc.scalar.copy(
                    vstage[:].rearrange("p (a h c) -> p a h c", a=16, h=4)[
                        :, tp * 4:(tp + 1) * 4, :, 0:64],
                    vacc[:].rearrange("p (a h c) -> p a h c", a=4, h=4))
            nc.scalar.dma_start(v_o[:, :], vstage[:])

    nc.compile()
    return nc


# ----------------------------------------------------------------- launch 2

def _build_l2():
    nc = bacc.Bacc("TRN2", target_bir_lowering=False, debug=False,
                   num_devices=NCORES)
    qR = nc.dram_tensor("qh", [GF, N], FP16, kind="ExternalInput")
    kR = nc.dram_tensor("kh", [GF, N], FP16, kind="ExternalInput")
    v_i = nc.dram_tensor("v65", [P, 16 * 260], FP16, kind="ExternalInput")
    wout_i = nc.dram_tensor("wout", [GF, DIN], FP16, kind="ExternalInput")
    part_o = nc.dram_tensor("part", [2, N, DIN], FP16, kind="ExternalOutput")

    IBW = 512        # query-block width
    NIB = N // IBW   # 4 query blocks
    NJT = N // P     # 16 key tiles

    with tile.TileContext(nc) as tc:
        with (
            tc.tile_pool(name="cst", bufs=1) as cst,
            tc.tile_pool(name="hatp", bufs=1) as hatp,
            tc.tile_pool(name="ptp", bufs=6) as ptp,
            tc.tile_pool(name="obig", bufs=1) as obigp,
            tc.tile_pool(name="onrm", bufs=2) as onrm,
            tc.tile_pool(name="outp", bufs=2) as outp,
            tc.tile_pool(name="tiny", bufs=4) as tiny,
            tc.tile_pool(name="psS", bufs=2, space="PSUM") as psS,
            tc.tile_pool(name="psA", bufs=1, space="PSUM") as psA,
            tc.tile_pool(name="psO", bufs=2, space="PSUM") as psO,
        ):
            # ---- loads: pair-0 tensors first so attention starts early ----
            khat, qhat = [], []
            for mt in range(2):
                kt_ = cst.tile([P, N], FP16, tag=f"kh{mt}")
                qt_ = cst.tile([P, N], FP16, tag=f"qh{mt}")
                khat.append(kt_)
                qhat.append(qt_)
            nc.sync.dma_start(khat[0][:], kR[0:P, :])
            nc.sync.dma_start(qhat[0][:], qR[0:P, :])
            vbig = cst.tile([P, 16 * 260], FP16, tag="v")
            nc.scalar.dma_start(vbig[:], v_i[:, :])
            nc.sync.dma_start(khat[1][:], kR[P:2 * P, :])
            nc.sync.dma_start(qhat[1][:], qR[P:2 * P, :])
            wout = []
            for kt in range(2):
                w = cst.tile([P, DIN], FP16, tag=f"wo{kt}")
                nc.scalar.dma_start(w[:], wout_i[kt * P:(kt + 1) * P, :])
                wout.append(w)

            def vt_slice(jt, h):
                return vbig[:, jt * 260 + h * 65:jt * 260 + (h + 1) * 65]

            # ---- attention ----
            obig = [obigp.tile([P, N], FP16, tag=f"obig{pr}", name=f"ob{pr}")
                    for pr in range(2)]

            osb_pend = {}

            def emit_proj(pr, tt, eng, use_act=False):
                pss = [psO.tile([P, 512], FP32, tag="O",
                                name=f"pj{pr}_{tt}_{hf}")
                       for hf in range(2)]
                for half in range(2):
                    nc.tensor.matmul(
                        pss[half][:],
                        obig[pr][:, tt * P:(tt + 1) * P],
                        wout[pr][:, half * 512:(half + 1) * 512],
                        start=True, stop=True,
                    )
                if tt % 2 == 0:
                    osb_pend[pr] = outp.tile([P, 2 * DIN], FP16, tag="osb",
                                             name=f"osb{pr}_{tt}")
                osb = osb_pend[pr]
                base = (tt % 2) * DIN
                if use_act:
                    nc.scalar.copy(osb[:, base:base + 512], pss[0][:])
                    nc.scalar.copy(osb[:, base + 512:base + 1024], pss[1][:])
                else:
                    eng.tensor_copy(osb[:, base:base + 512], pss[0][:])
                    eng.tensor_copy(osb[:, base + 512:base + 1024], pss[1][:])
                if tt % 2 == 1:
                    nc.sync.dma_start(
                        part_o[pr, (tt - 1) * P:(tt + 1) * P, :].rearrange(
                            "(a p) d -> p a d", p=P),
                        osb[:].rearrange("p (a d) -> p a d", a=2))

            def finish_block(pr, ib, o_ps):
                rinv, bc = [], []
                for sub in range(2):
                    d = tiny.tile([1, IBW], FP32, tag="dsb",
                                  name=f"d{pr}_{ib}_{sub}")
                    nc.vector.tensor_copy(d[:, :], o_ps[sub][64:65, :])
                    t = tiny.tile([1, IBW], FP32, tag="rinv",
                                  name=f"ri{pr}_{ib}_{sub}")
                    nc.vector.reciprocal_approx_fast(t[:, :], d[:, :])
                    rinv.append(t)
                for sub in range(2):
                    t = tiny.tile([64, IBW], FP32, tag="bc",
                                  name=f"bc{pr}_{ib}_{sub}")
                    nc.gpsimd.partition_broadcast(t[:, :], rinv[sub][:, :])
                    bc.append(t)
                nc.vector.tensor_mul(
                    obig[pr][0:64, ib * IBW:(ib + 1) * IBW],
                    o_ps[0][0:64, :], bc[0][:, :])
                onr = onrm.tile([64, IBW], FP16, tag="onr")
                nc.vector.tensor_mul(onr[:, :], o_ps[1][0:64, :], bc[1][:, :])
                nc.sync.dma_start(
                    obig[pr][64:128, ib * IBW:(ib + 1) * IBW], onr[:, :])

            steps = [(pr, ib, jt) for pr in range(2) for ib in range(NIB)
                     for jt in range(NJT)]
            p_sbs = {}
            o_ps_map = {}
            ready = []      # projection token-tiles whose obig cols are done
            launched = []

            def emit_s(step):
                pr, ib, jt = step
                s_ps = psS.tile([P, 2 * IBW], FP32, tag="S")
                for sub in range(2):
                    nc.tensor.matmul(
                        s_ps[:, sub * IBW:(sub + 1) * IBW],
                        khat[pr][sub * 64:(sub + 1) * 64, jt * P:(jt + 1) * P],
                        qhat[pr][sub * 64:(sub + 1) * 64,
                                 ib * IBW:(ib + 1) * IBW],
                        start=True, stop=True,
                        tile_position=(64 * sub, 0),
                    )
                p_sb = ptp.tile([P, 2 * IBW], FP16, tag="P",
                                name=f"p{pr}_{ib}_{jt}")
                nc.scalar.activation(p_sb[:, :], s_ps[:, :],
                                     AF.Exp, scale=0.125)
                p_sbs[step] = p_sb

            nproj = 0
            for k in range(4):
                emit_s(steps[k])
            for si, step in enumerate(steps):
                pr, ib, jt = step
                if si + 4 < len(steps):
                    emit_s(steps[si + 4])
                if (pr, ib) not in o_ps_map:
                    o_ps_map[(pr, ib)] = [
                        psA.tile([65, IBW], FP32, tag=f"oacc{s}",
                                 name=f"o{pr}_{ib}_{s}") for s in range(2)]
                o_ps = o_ps_map[(pr, ib)]
                p_sb = p_sbs.pop(step)
                for sub in range(2):
                    h = 2 * pr + sub
                    nc.tensor.matmul(
                        o_ps[sub][:, :],
                        vt_slice(jt, h),
                        p_sb[:, sub * IBW:(sub + 1) * IBW],
                        start=(jt == 0), stop=(jt == NJT - 1),
                    )
                if jt in (1, 4, 7, 10) and ready:
                    ptt = ready.pop(0)
                    emit_proj(ptt[0], ptt[1], nc.vector)
                    nproj += 1
                if jt == NJT - 1:
                    finish_block(pr, ib, o_ps)
                    ready.extend((pr, ib * 4 + i) for i in range(4))

            for ptt in ready:
                emit_proj(ptt[0], ptt[1], nc.vector, use_act=(nproj % 2 == 0))
                nproj += 1

    nc.compile()
    return nc


# ------------------------------------------------------------------- driver

def _rope_tables():
    half = DH // 2
    inv_freq = 1.0 / (ROPE_BASE ** (np.arange(half, dtype=np.float64) * 2.0
                                    / DH))
    freqs = np.arange(N, dtype=np.float64)[:, None] * inv_freq[None, :]
    cos = np.cos(freqs).T          # (32, N)
    sin = np.sin(freqs).T
    cos64 = np.concatenate([cos, cos], 0)            # (64, N)
    sin64 = np.concatenate([-sin, sin], 0)           # signed for rotate_half
    cos_t = np.ascontiguousarray(
        np.concatenate([cos64, cos64], 0).astype(np.float16))  # (128, N)
    sin_t = np.ascontiguousarray(
        np.concatenate([sin64, sin64], 0).astype(np.float16))
    return cos_t, sin_t


def kernel(input, w_qkv, b_qkv, q_scale, k_scale, w_out, b_out):
    trace = bool(os.environ.get("KERNEL_TRACE"))
    if "l1" not in _cache:
        _cache["l1"] = _build_l1()
    if "l2" not in _cache:
        _cache["l2"] = _build_l2()

    x = np.asarray(input, dtype=np.float32)
    w_qkv = np.asarray(w_qkv, dtype=np.float32)
    b_qkv = np.asarray(b_qkv, dtype=np.float32)
    qs = np.asarray(q_scale, dtype=np.float32)
    ks = np.asarray(k_scale, dtype=np.float32)
    w_out = np.asarray(w_out, dtype=np.float32)
    b_out = np.asarray(b_out, dtype=np.float32)

    wq = w_qkv[:, :DQ] * qs[None, :]
    wk = w_qkv[:, DQ:2 * DQ] * ks[None, :]
    wv = w_qkv[:, 2 * DQ:]
    bq = b_qkv[:DQ] * qs
    bk = b_qkv[DQ:2 * DQ] * ks
    bv = b_qkv[2 * DQ:]

    xT = [np.ascontiguousarray(x[b].T.astype(np.float16)) for b in range(B)]
    cos_t, sin_t = _rope_tables()

    def col4(vec256_a, vec256_b):
        # -> (128, 4): [a_mt0 | a_mt1 | b_mt0 | b_mt1]
        return np.ascontiguousarray(np.stack(
            [vec256_a[:P], vec256_a[P:], vec256_b[:P], vec256_b[P:]],
            axis=1).astype(np.float32))

    in1 = []
    for c in range(NCORES):
        b, g = divmod(c, NGROUP)
        sl = slice(g * GF, (g + 1) * GF)
        wcat = np.ascontiguousarray(np.concatenate(
            [wq[:, sl], wk[:, sl], wv[:, sl]], axis=1).astype(np.float16))
        in1.append({
            "xT": xT[b],
            "wcat": wcat,
            "bqk": col4(bq[sl], bk[sl]),
            "invs": np.ascontiguousarray(np.repeat(
                col4(1.0 / np.square(qs[sl]), 1.0 / np.square(ks[sl])),
                32, axis=1).astype(np.float16)),
            "cosr": cos_t,
            "sinr": sin_t,
        })

    r1 = run_bass_kernel_spmd(_cache["l1"], in1,
                              core_ids=list(range(NCORES)), trace=trace)
    if trace:
        LAST_EXEC_NS["l1"] = r1.exec_time_ns
        LAST_RESULTS["l1"] = r1

    # host: combine partial ssq -> rsqrt factors
    tabs = {}
    for b in range(B):
        sq_q = np.zeros(N, np.float64)
        sq_k = np.zeros(N, np.float64)
        for g in range(NGROUP):
            ssq = r1.results[NGROUP * b + g]["ssq"].astype(np.float64)
            sq_q += ssq[0]
            sq_k += ssq[1]
        tabs[b] = {
            "r_q": (1.0 / np.sqrt(sq_q / DQ + EPS)).astype(np.float32),
            "r_k": (1.0 / np.sqrt(sq_k / DQ + EPS)).astype(np.float32),
        }

    in2 = []
    for c in range(NCORES):
        b, g = divmod(c, NGROUP)
        sl = slice(g * GF, (g + 1) * GF)
        in2.append({
            "qh": np.ascontiguousarray(
                r1.results[c]["qR"].astype(np.float32)
                * tabs[b]["r_q"][None, :]).astype(np.float16),
            "kh": np.ascontiguousarray(
                r1.results[c]["kR"].astype(np.float32)
                * tabs[b]["r_k"][None, :]).astype(np.float16),
            "v65": r1.results[c]["v65"],
            "wout": np.ascontiguousarray(w_out[sl, :].astype(np.float16)),
        })

    r2 = run_bass_kernel_spmd(_cache["l2"], in2,
                              core_ids=list(range(NCORES)), trace=trace)
    if trace:
        LAST_EXEC_NS["l2"] = r2.exec_time_ns
        LAST_RESULTS["l2"] = r2

    base = (bv.astype(np.float64) @ w_out.astype(np.float64)
            + b_out.astype(np.float64))
    out = np.zeros((B, N, DIN), np.float32)
    for b in range(B):
        acc = np.zeros((N, DIN), np.float64)
        for g in range(NGROUP):
            p = r2.results[NGROUP * b + g]["part"].astype(np.float64)
            acc += p[0]
            acc += p[1]
        out[b] = (acc + base[None, :]).astype(np.float32)
    return out


# revision 17
# speedup vs baseline: 1.1001x; 1.0334x over previous
"""DiT attention block on 8 Trainium2 NeuronCores.

Reference computation (fp32):
    qkv = x @ Wqkv + b            (b=2, n=2048, din=1024, 3*1024)
    q, k = RMSNorm_full_dim(q|k) * scale  (norm over all 1024 channels)
    RoPE (rotary_dim=64) per 64-dim head, 16 heads
    attn = softmax(q k^T / 8) v ;  out = attn @ Wout + bout
    Sharding: 8 cores = 2 batches x 4 head-groups (4 heads / 256 features).

Two SPMD launches (fp16 data paths, fp32 accumulation):
  L1: qkv projection in transposed layout, kt-streamed so the PE computes
      behind the input DMA stream; RoPE applied on-chip (rotation commutes
      with the norm scale); partial sum-of-squares for the full-dim RMSNorm
      (weighted by 1/scale^2, so it must read the PRE-rope values); V packed
      [v_h|1] per head on-chip.  Host combines ssq -> rsqrt factors.
  L2: qhat = qR * broadcast(r_q) (host-materialized broadcast); khat = kR
      with r_k folded into the exp's per-partition scale operand.  Attention
      stream: S^T = kR^T qhat (PE, row-tiled head pairs), exp straight from
      PSUM (ACT, the bottleneck engine: ~1.09us per [128,1024] step), O^T
      = [V|1]^T P.  Normalization uses reciprocal_approx_fast + gpsimd
      partition_broadcast.  Out-projection partials are injected densely
      into the attention stream; host adds the partials + bias term.
"""

import os
import sys

for _p in ("/opt/trn_rl_repo", "/root/.axon_site/_ro/trn_rl_repo"):
    if os.path.isdir(_p) and _p not in sys.path:
        sys.path.append(_p)

import numpy as np

import concourse.bass as bass  # noqa: E402,F401
import concourse.mybir as mybir  # noqa: E402
import concourse.tile as tile  # noqa: E402
from concourse import bacc  # noqa: E402
from concourse.bass_utils import run_bass_kernel_spmd  # noqa: E402

FP32 = mybir.dt.float32
FP16 = mybir.dt.float16
AF = mybir.ActivationFunctionType

B = 2
N = 2048
DIN = 1024
DQ = 1024
H = 16
DH = 64
NCORES = 8
NGROUP = 4          # head-groups per batch
GF = 256            # features per core (4 heads)
P = 128
EPS = 1e-6
ROPE_BASE = 10000.0

LAST_EXEC_NS = {}   # filled when KERNEL_TRACE=1
LAST_RESULTS = {}   # BassKernelResults per launch when KERNEL_TRACE=1

_cache = {}


# ----------------------------------------------------------------- launch 1

def _build_l1():
    nc = bacc.Bacc("TRN2", target_bir_lowering=False, debug=False,
                   num_devices=NCORES)
    xT = nc.dram_tensor("xT", [DIN, N], FP16, kind="ExternalInput")
    wcat = nc.dram_tensor("wcat", [DIN, 3 * GF], FP16, kind="ExternalInput")
    bqk = nc.dram_tensor("bqk", [P, 4], FP32, kind="ExternalInput")
    invs = nc.dram_tensor("invs", [P, P], FP16, kind="ExternalInput")
    cosr = nc.dram_tensor("cosr", [P, N], FP16, kind="ExternalInput")
    sinr = nc.dram_tensor("sinr", [P, N], FP16, kind="ExternalInput")
    qR_o = nc.dram_tensor("qR", [GF, N], FP16, kind="ExternalOutput")
    kR_o = nc.dram_tensor("kR", [GF, N], FP16, kind="ExternalOutput")
    # v65 layout: token-major rows, per key-tile column groups of 4*(64+1)
    v_o = nc.dram_tensor("v65", [P, 16 * 260], FP16, kind="ExternalOutput")
    ssq_o = nc.dram_tensor("ssq", [2, N], FP32, kind="ExternalOutput")

    KT = DIN // P  # 8 contraction tiles

    with tile.TileContext(nc) as tc:
        with (
            tc.tile_pool(name="xw", bufs=1) as xw,
            tc.tile_pool(name="bigp", bufs=1) as bigp,
            tc.tile_pool(name="scr", bufs=2) as scr,
            tc.tile_pool(name="shp", bufs=2) as shp,
            tc.tile_pool(name="outq", bufs=2) as outq,
            tc.tile_pool(name="sqp", bufs=4) as sqp,
            tc.tile_pool(name="vst", bufs=1) as vst,
            tc.tile_pool(name="stgp", bufs=1) as stgp,
            tc.tile_pool(name="ps", bufs=4, space="PSUM") as ps,
        ):
            # ---- input DMAs: xt on sync queue, wt on scalar queue ----
            xt, wt = [], []
            for kt in range(KT):
                t = xw.tile([P, N], FP16, tag=f"xt{kt}")
                w = xw.tile([P, 3 * GF], FP16, tag=f"wt{kt}")
                if kt == 0:
                    nc.scalar.dma_start(w[:, 0:GF], wcat[0:P, 0:GF])
                    nc.sync.dma_start(t[:, 0:1024], xT[0:P, 0:1024])
                    nc.sync.dma_start(t[:, 1024:2048], xT[0:P, 1024:2048])
                    nc.scalar.dma_start(w[:, GF:3 * GF], wcat[0:P, GF:3 * GF])
                else:
                    nc.sync.dma_start(t[:], xT[kt * P:(kt + 1) * P, :])
                    nc.scalar.dma_start(w[:], wcat[kt * P:(kt + 1) * P, :])
                xt.append(t)
                wt.append(w)
            bias = xw.tile([P, 4], FP32, tag="bias")
            nc.scalar.dma_start(bias[:], bqk[:, :])
            winv = xw.tile([P, P], FP16, tag="winv")
            nc.scalar.dma_start(winv[:], invs[:, :])
            cosb = xw.tile([P, N], FP16, tag="cos")
            nc.scalar.dma_start(cosb[:], cosr[:, :])
            sinb = xw.tile([P, N], FP16, tag="sin")
            nc.scalar.dma_start(sinb[:], sinr[:, :])

            # v staging: 16 key-tiles x (4 heads x 65); ones columns set once
            vstage = vst.tile([P, 16 * 260], FP16, tag="vstage")
            ones = vst.tile([P, 64], FP16, tag="ones")
            nc.vector.memset(ones[:], 1.0)
            nc.vector.tensor_copy(
                vstage[:].rearrange("p (a h c) -> p a h c", a=16, h=4)[
                    :, :, :, 64:65],
                ones[:].rearrange("p (a h c) -> p a h c", a=16, h=4))

            sq = {}

            def qk_phase(t_idx, out_dram, streamed):
                # projection for one of q/k in transposed layout.
                col0 = t_idx * GF
                accs = [ps.tile([P, 1024], FP32, tag="acc",
                                name=f"acc{t_idx}_{i}") for i in range(4)]
                if streamed:
                    # kt-outer: compute behind the input DMA stream
                    for kt in range(KT):
                        for mt in range(2):
                            for nb in range(4):
                                nc.tensor.matmul(
                                    accs[mt * 2 + nb // 2][
                                        :, (nb % 2) * 512:(nb % 2 + 1) * 512],
                                    wt[kt][:, col0 + mt * P:col0 + (mt + 1) * P],
                                    xt[kt][:, nb * 512:(nb + 1) * 512],
                                    start=(kt == 0), stop=(kt == KT - 1),
                                )
                else:
                    for mt in range(2):
                        for nb in range(4):
                            for kt in range(KT):
                                nc.tensor.matmul(
                                    accs[mt * 2 + nb // 2][
                                        :, (nb % 2) * 512:(nb % 2 + 1) * 512],
                                    wt[kt][:, col0 + mt * P:col0 + (mt + 1) * P],
                                    xt[kt][:, nb * 512:(nb + 1) * 512],
                                    start=(kt == 0), stop=(kt == KT - 1),
                                )
                dmae = nc.scalar if t_idx == 0 else nc.sync
                for mt in range(2):
                    big = bigp.tile([P, N], FP16, tag=f"big{t_idx}_{mt}")
                    s = sqp.tile([P, N], FP16, tag=f"sq{t_idx}_{mt}")
                    for nbp in range(2):
                        nc.scalar.activation(
                            big[:, nbp * 1024:(nbp + 1) * 1024],
                            accs[mt * 2 + nbp][:], AF.Identity,
                            bias=bias[:, 2 * t_idx + mt:2 * t_idx + mt + 1])
                        # pre-rope squares for the weighted ssq, fused as
                        # (acc + bias)^2 on the ACT engine (idle here)
                        nc.scalar.activation(
                            s[:, nbp * 1024:(nbp + 1) * 1024],
                            accs[mt * 2 + nbp][:], AF.Square,
                            bias=bias[:, 2 * t_idx + mt:2 * t_idx + mt + 1])
                    sq[(t_idx, mt)] = s
                    # rope: rotate_half via 4 partition-block DMAs
                    sh = shp.tile([P, N], FP16, tag="sh")
                    for blk in range(4):
                        srcb = blk ^ 1
                        (dmae if blk < 2 else nc.gpsimd).dma_start(
                            sh[blk * 32:(blk + 1) * 32, :],
                            big[srcb * 32:(srcb + 1) * 32, :])
                    t2 = scr.tile([P, N], FP16, tag="t2")
                    nc.vector.tensor_mul(t2[:], big[:], cosb[:])
                    nc.vector.tensor_mul(sh[:], sh[:], sinb[:])
                    rr = outq.tile([P, N], FP16, tag="rr")
                    nc.vector.tensor_add(rr[:], t2[:], sh[:])
                    dmae.dma_start(out_dram[mt * P:(mt + 1) * P, :], rr[:])

            qk_phase(0, qR_o, streamed=True)
            qk_phase(1, kR_o, streamed=False)

            # ---- ssq: 32 identical output rows via all-equal lhsT columns ----
            stg = stgp.tile([1, 2 * N], FP32, tag="stg")
            for t_idx in range(2):
                for np2 in range(2):
                    sp = ps.tile([32, 1024], FP32, tag="acc",
                                 name=f"ssq{t_idx}_{np2}")
                    for nbi in range(2):
                        nb = np2 * 2 + nbi
                        for mt in range(2):
                            nc.tensor.matmul(
                                sp[:, nbi * 512:(nbi + 1) * 512],
                                winv[:, 32 * (2 * t_idx + mt):
                                     32 * (2 * t_idx + mt + 1)],
                                sq[(t_idx, mt)][:, nb * 512:(nb + 1) * 512],
                                start=(mt == 0), stop=(mt == 1),
                            )
                    nc.scalar.copy(
                        stg[0:1, t_idx * N + np2 * 1024:
                            t_idx * N + (np2 + 1) * 1024],
                        sp[0:1, :])
            for t_idx in range(2):
                nc.sync.dma_start(ssq_o[t_idx:t_idx + 1, :],
                                  stg[0:1, t_idx * N:(t_idx + 1) * N])

            # ---- v phase (tiles resident now) ----
            for tp in range(4):
                vacc = ps.tile([P, 1024], FP32, tag="acc", name=f"vacc{tp}")
                for ti in range(4):
                    tt = tp * 4 + ti
                    for kt in range(KT):
                        nc.tensor.matmul(
                            vacc[:, ti * 256:(ti + 1) * 256],
                            xt[kt][:, tt * P:(tt + 1) * P],
                            wt[kt][:, 2 * GF:3 * GF],
                            start=(kt == 0), stop=(kt == KT - 1),
                        )
                nc.scalar.copy(
                    vstage[:].rearrange("p (a h c) -> p a h c", a=16, h=4)[
                        :, tp * 4:(tp + 1) * 4, :, 0:64],
                    vacc[:].rearrange("p (a h c) -> p a h c", a=4, h=4))
            nc.scalar.dma_start(v_o[:, :], vstage[:])

    nc.compile()
    return nc


# ----------------------------------------------------------------- launch 2

def _build_l2():
    nc = bacc.Bacc("TRN2", target_bir_lowering=False, debug=False,
                   num_devices=NCORES)
    qR = nc.dram_tensor("qh", [GF, N], FP16, kind="ExternalInput")
    kR = nc.dram_tensor("kh", [GF, N], FP16, kind="ExternalInput")
    v_i = nc.dram_tensor("v65", [P, 16 * 260], FP16, kind="ExternalInput")
    wout_i = nc.dram_tensor("wout", [GF, DIN], FP16, kind="ExternalInput")
    part_o = nc.dram_tensor("part", [2, N, DIN], FP16, kind="ExternalOutput")

    IBW = 512        # query-block width
    NIB = N // IBW   # 4 query blocks
    NJT = N // P     # 16 key tiles

    with tile.TileContext(nc) as tc:
        with (
            tc.tile_pool(name="cst", bufs=1) as cst,
            tc.tile_pool(name="hatp", bufs=1) as hatp,
            tc.tile_pool(name="ptp", bufs=8) as ptp,
            tc.tile_pool(name="obig", bufs=1) as obigp,
            tc.tile_pool(name="onrm", bufs=2) as onrm,
            tc.tile_pool(name="outp", bufs=2) as outp,
            tc.tile_pool(name="tiny", bufs=4) as tiny,
            tc.tile_pool(name="psS", bufs=2, space="PSUM") as psS,
            tc.tile_pool(name="psA", bufs=1, space="PSUM") as psA,
            tc.tile_pool(name="psO", bufs=2, space="PSUM") as psO,
        ):
            # ---- loads: pair-0 tensors first so attention starts early ----
            khat, qhat = [], []
            for mt in range(2):
                kt_ = cst.tile([P, N], FP16, tag=f"kh{mt}")
                qt_ = cst.tile([P, N], FP16, tag=f"qh{mt}")
                khat.append(kt_)
                qhat.append(qt_)
            # first S step needs only kh0[:, :512] + qh0[:, :512]; lead with
            # small chunks split across both DMA queues, stream v in jt-chunks
            nc.sync.dma_start(khat[0][:, 0:512], kR[0:P, 0:512])
            nc.scalar.dma_start(qhat[0][:, 0:512], qR[0:P, 0:512])
            nc.sync.dma_start(khat[0][:, 512:N], kR[0:P, 512:N])
            nc.scalar.dma_start(qhat[0][:, 512:N], qR[0:P, 512:N])
            vbig = cst.tile([P, 16 * 260], FP16, tag="v")
            for vc in range(4):
                nc.sync.dma_start(vbig[:, vc * 1040:(vc + 1) * 1040],
                                  v_i[:, vc * 1040:(vc + 1) * 1040])
            wout = []
            for kt in range(2):
                w = cst.tile([P, DIN], FP16, tag=f"wo{kt}")
                nc.scalar.dma_start(w[:], wout_i[kt * P:(kt + 1) * P, :])
                wout.append(w)
            nc.scalar.dma_start(qhat[1][:], qR[P:2 * P, :])
            nc.sync.dma_start(khat[1][:], kR[P:2 * P, :])

            def vt_slice(jt, h):
                return vbig[:, jt * 260 + h * 65:jt * 260 + (h + 1) * 65]

            # ---- attention ----
            obig = [obigp.tile([P, N], FP16, tag=f"obig{pr}", name=f"ob{pr}")
                    for pr in range(2)]

            osb_pend = {}

            def emit_proj(pr, tt, eng, use_act=False):
                pss = [psO.tile([P, 512], FP32, tag="O",
                                name=f"pj{pr}_{tt}_{hf}")
                       for hf in range(2)]
                for half in range(2):
                    nc.tensor.matmul(
                        pss[half][:],
                        obig[pr][:, tt * P:(tt + 1) * P],
                        wout[pr][:, half * 512:(half + 1) * 512],
                        start=True, stop=True,
                    )
                if tt % 2 == 0:
                    osb_pend[pr] = outp.tile([P, 2 * DIN], FP16, tag="osb",
                                             name=f"osb{pr}_{tt}")
                osb = osb_pend[pr]
                base = (tt % 2) * DIN
                if use_act:
                    nc.scalar.copy(osb[:, base:base + 512], pss[0][:])
                    nc.scalar.copy(osb[:, base + 512:base + 1024], pss[1][:])
                else:
                    eng.tensor_copy(osb[:, base:base + 512], pss[0][:])
                    eng.tensor_copy(osb[:, base + 512:base + 1024], pss[1][:])
                if tt % 2 == 1:
                    nc.sync.dma_start(
                        part_o[pr, (tt - 1) * P:(tt + 1) * P, :].rearrange(
                            "(a p) d -> p a d", p=P),
                        osb[:].rearrange("p (a d) -> p a d", a=2))

            def finish_block(pr, ib, o_ps):
                rinv, bc = [], []
                for sub in range(2):
                    d = tiny.tile([1, IBW], FP32, tag="dsb",
                                  name=f"d{pr}_{ib}_{sub}")
                    nc.vector.tensor_copy(d[:, :], o_ps[sub][64:65, :])
                    t = tiny.tile([1, IBW], FP32, tag="rinv",
                                  name=f"ri{pr}_{ib}_{sub}")
                    nc.vector.reciprocal_approx_fast(t[:, :], d[:, :])
                    rinv.append(t)
                for sub in range(2):
                    t = tiny.tile([64, IBW], FP32, tag="bc",
                                  name=f"bc{pr}_{ib}_{sub}")
                    nc.gpsimd.partition_broadcast(t[:, :], rinv[sub][:, :])
                    bc.append(t)
                nc.vector.tensor_mul(
                    obig[pr][0:64, ib * IBW:(ib + 1) * IBW],
                    o_ps[0][0:64, :], bc[0][:, :])
                onr = onrm.tile([64, IBW], FP16, tag="onr")
                nc.vector.tensor_mul(onr[:, :], o_ps[1][0:64, :], bc[1][:, :])
                nc.sync.dma_start(
                    obig[pr][64:128, ib * IBW:(ib + 1) * IBW], onr[:, :])

            steps = [(pr, ib, jt) for pr in range(2) for ib in range(NIB)
                     for jt in range(NJT)]
            p_sbs = {}
            o_ps_map = {}
            ready = []      # projection token-tiles whose obig cols are done
            launched = []

            def emit_s(step):
                pr, ib, jt = step
                s_ps = psS.tile([P, 2 * IBW], FP32, tag="S")
                for sub in range(2):
                    nc.tensor.matmul(
                        s_ps[:, sub * IBW:(sub + 1) * IBW],
                        khat[pr][sub * 64:(sub + 1) * 64, jt * P:(jt + 1) * P],
                        qhat[pr][sub * 64:(sub + 1) * 64,
                                 ib * IBW:(ib + 1) * IBW],
                        start=True, stop=True,
                        tile_position=(64 * sub, 0),
                    )
                p_sb = ptp.tile([P, 2 * IBW], FP16, tag="P",
                                name=f"p{pr}_{ib}_{jt}")
                nc.scalar.activation(p_sb[:, :], s_ps[:, :],
                                     AF.Exp, scale=0.125)
                p_sbs[step] = p_sb

            nproj = 0
            for k in range(6):
                emit_s(steps[k])
            for si, step in enumerate(steps):
                pr, ib, jt = step
                if si + 6 < len(steps):
                    emit_s(steps[si + 6])
                if (pr, ib) not in o_ps_map:
                    o_ps_map[(pr, ib)] = [
                        psA.tile([65, IBW], FP32, tag=f"oacc{s}",
                                 name=f"o{pr}_{ib}_{s}") for s in range(2)]
                o_ps = o_ps_map[(pr, ib)]
                p_sb = p_sbs.pop(step)
                for sub in range(2):
                    h = 2 * pr + sub
                    nc.tensor.matmul(
                        o_ps[sub][:, :],
                        vt_slice(jt, h),
                        p_sb[:, sub * IBW:(sub + 1) * IBW],
                        start=(jt == 0), stop=(jt == NJT - 1),
                    )
                if jt in (1, 4, 7, 10) and ready:
                    ptt = ready.pop(0)
                    emit_proj(ptt[0], ptt[1], nc.vector)
                    nproj += 1
                if jt == NJT - 1:
                    finish_block(pr, ib, o_ps)
                    ready.extend((pr, ib * 4 + i) for i in range(4))

            for ptt in ready:
                emit_proj(ptt[0], ptt[1], nc.vector, use_act=(nproj % 2 == 0))
                nproj += 1

    nc.compile()
    return nc


# ------------------------------------------------------------------- driver

def _rope_tables():
    half = DH // 2
    inv_freq = 1.0 / (ROPE_BASE ** (np.arange(half, dtype=np.float64) * 2.0
                                    / DH))
    freqs = np.arange(N, dtype=np.float64)[:, None] * inv_freq[None, :]
    cos = np.cos(freqs).T          # (32, N)
    sin = np.sin(freqs).T
    cos64 = np.concatenate([cos, cos], 0)            # (64, N)
    sin64 = np.concatenate([-sin, sin], 0)           # signed for rotate_half
    cos_t = np.ascontiguousarray(
        np.concatenate([cos64, cos64], 0).astype(np.float16))  # (128, N)
    sin_t = np.ascontiguousarray(
        np.concatenate([sin64, sin64], 0).astype(np.float16))
    return cos_t, sin_t


def kernel(input, w_qkv, b_qkv, q_scale, k_scale, w_out, b_out):
    trace = bool(os.environ.get("KERNEL_TRACE"))
    if "l1" not in _cache:
        _cache["l1"] = _build_l1()
    if "l2" not in _cache:
        _cache["l2"] = _build_l2()

    x = np.asarray(input, dtype=np.float32)
    w_qkv = np.asarray(w_qkv, dtype=np.float32)
    b_qkv = np.asarray(b_qkv, dtype=np.float32)
    qs = np.asarray(q_scale, dtype=np.float32)
    ks = np.asarray(k_scale, dtype=np.float32)
    w_out = np.asarray(w_out, dtype=np.float32)
    b_out = np.asarray(b_out, dtype=np.float32)

    wq = w_qkv[:, :DQ] * qs[None, :]
    wk = w_qkv[:, DQ:2 * DQ] * ks[None, :]
    wv = w_qkv[:, 2 * DQ:]
    bq = b_qkv[:DQ] * qs
    bk = b_qkv[DQ:2 * DQ] * ks
    bv = b_qkv[2 * DQ:]

    xT = [np.ascontiguousarray(x[b].T.astype(np.float16)) for b in range(B)]
    cos_t, sin_t = _rope_tables()

    def col4(vec256_a, vec256_b):
        # -> (128, 4): [a_mt0 | a_mt1 | b_mt0 | b_mt1]
        return np.ascontiguousarray(np.stack(
            [vec256_a[:P], vec256_a[P:], vec256_b[:P], vec256_b[P:]],
            axis=1).astype(np.float32))

    in1 = []
    for c in range(NCORES):
        b, g = divmod(c, NGROUP)
        sl = slice(g * GF, (g + 1) * GF)
        wcat = np.ascontiguousarray(np.concatenate(
            [wq[:, sl], wk[:, sl], wv[:, sl]], axis=1).astype(np.float16))
        in1.append({
            "xT": xT[b],
            "wcat": wcat,
            "bqk": col4(bq[sl], bk[sl]),
            "invs": np.ascontiguousarray(np.repeat(
                col4(1.0 / np.square(qs[sl]), 1.0 / np.square(ks[sl])),
                32, axis=1).astype(np.float16)),
            "cosr": cos_t,
            "sinr": sin_t,
        })

    r1 = run_bass_kernel_spmd(_cache["l1"], in1,
                              core_ids=list(range(NCORES)), trace=trace)
    if trace:
        LAST_EXEC_NS["l1"] = r1.exec_time_ns
        LAST_RESULTS["l1"] = r1

    # host: combine partial ssq -> rsqrt factors
    tabs = {}
    for b in range(B):
        sq_q = np.zeros(N, np.float64)
        sq_k = np.zeros(N, np.float64)
        for g in range(NGROUP):
            ssq = r1.results[NGROUP * b + g]["ssq"].astype(np.float64)
            sq_q += ssq[0]
            sq_k += ssq[1]
        tabs[b] = {
            "r_q": (1.0 / np.sqrt(sq_q / DQ + EPS)).astype(np.float32),
            "r_k": (1.0 / np.sqrt(sq_k / DQ + EPS)).astype(np.float32),
        }

    in2 = []
    for c in range(NCORES):
        b, g = divmod(c, NGROUP)
        sl = slice(g * GF, (g + 1) * GF)
        in2.append({
            "qh": np.ascontiguousarray(
                r1.results[c]["qR"].astype(np.float32)
                * tabs[b]["r_q"][None, :]).astype(np.float16),
            "kh": np.ascontiguousarray(
                r1.results[c]["kR"].astype(np.float32)
                * tabs[b]["r_k"][None, :]).astype(np.float16),
            "v65": r1.results[c]["v65"],
            "wout": np.ascontiguousarray(w_out[sl, :].astype(np.float16)),
        })

    r2 = run_bass_kernel_spmd(_cache["l2"], in2,
                              core_ids=list(range(NCORES)), trace=trace)
    if trace:
        LAST_EXEC_NS["l2"] = r2.exec_time_ns
        LAST_RESULTS["l2"] = r2

    base = (bv.astype(np.float64) @ w_out.astype(np.float64)
            + b_out.astype(np.float64))
    out = np.zeros((B, N, DIN), np.float32)
    for b in range(B):
        acc = np.zeros((N, DIN), np.float64)
        for g in range(NGROUP):
            p = r2.results[NGROUP * b + g]["part"].astype(np.float64)
            acc += p[0]
            acc += p[1]
        out[b] = (acc + base[None, :]).astype(np.float32)
    return out


# revision 18
# speedup vs baseline: 1.1082x; 1.0074x over previous
"""DiT attention block on 8 Trainium2 NeuronCores.

Reference computation (fp32):
    qkv = x @ Wqkv + b            (b=2, n=2048, din=1024, 3*1024)
    q, k = RMSNorm_full_dim(q|k) * scale  (norm over all 1024 channels)
    RoPE (rotary_dim=64) per 64-dim head, 16 heads
    attn = softmax(q k^T / 8) v ;  out = attn @ Wout + bout
    Sharding: 8 cores = 2 batches x 4 head-groups (4 heads / 256 features).

Two SPMD launches (fp16 data paths, fp32 accumulation):
  L1: qkv projection in transposed layout, kt-streamed so the PE computes
      behind the input DMA stream; RoPE applied on-chip (rotation commutes
      with the norm scale); partial sum-of-squares for the full-dim RMSNorm
      (weighted by 1/scale^2, so it must read the PRE-rope values); V packed
      [v_h|1] per head on-chip.  Host combines ssq -> rsqrt factors.
  L2: qhat = qR * broadcast(r_q) (host-materialized broadcast); khat = kR
      with r_k folded into the exp's per-partition scale operand.  Attention
      stream: S^T = kR^T qhat (PE, row-tiled head pairs), exp straight from
      PSUM (ACT, the bottleneck engine: ~1.09us per [128,1024] step), O^T
      = [V|1]^T P.  Normalization uses reciprocal_approx_fast + gpsimd
      partition_broadcast.  Out-projection partials are injected densely
      into the attention stream; host adds the partials + bias term.
"""

import os
import sys

for _p in ("/opt/trn_rl_repo", "/root/.axon_site/_ro/trn_rl_repo"):
    if os.path.isdir(_p) and _p not in sys.path:
        sys.path.append(_p)

import numpy as np

import concourse.bass as bass  # noqa: E402,F401
import concourse.mybir as mybir  # noqa: E402
import concourse.tile as tile  # noqa: E402
from concourse import bacc  # noqa: E402
from concourse.bass_utils import run_bass_kernel_spmd  # noqa: E402

FP32 = mybir.dt.float32
FP16 = mybir.dt.float16
AF = mybir.ActivationFunctionType

B = 2
N = 2048
DIN = 1024
DQ = 1024
H = 16
DH = 64
NCORES = 8
NGROUP = 4          # head-groups per batch
GF = 256            # features per core (4 heads)
P = 128
EPS = 1e-6
ROPE_BASE = 10000.0

LAST_EXEC_NS = {}   # filled when KERNEL_TRACE=1
LAST_RESULTS = {}   # BassKernelResults per launch when KERNEL_TRACE=1

_cache = {}


# ----------------------------------------------------------------- launch 1

def _build_l1():
    nc = bacc.Bacc("TRN2", target_bir_lowering=False, debug=False,
                   num_devices=NCORES)
    xT = nc.dram_tensor("xT", [DIN, N], FP16, kind="ExternalInput")
    wcat = nc.dram_tensor("wcat", [DIN, 3 * GF], FP16, kind="ExternalInput")
    bqk = nc.dram_tensor("bqk", [P, 4], FP32, kind="ExternalInput")
    invs = nc.dram_tensor("invs", [P, P], FP16, kind="ExternalInput")
    cosr = nc.dram_tensor("cosr", [P, N], FP16, kind="ExternalInput")
    sinr = nc.dram_tensor("sinr", [P, N], FP16, kind="ExternalInput")
    qR_o = nc.dram_tensor("qR", [GF, N], FP16, kind="ExternalOutput")
    kR_o = nc.dram_tensor("kR", [GF, N], FP16, kind="ExternalOutput")
    # v65 layout: token-major rows, per key-tile column groups of 4*(64+1)
    v_o = nc.dram_tensor("v65", [P, 16 * 260], FP16, kind="ExternalOutput")
    ssq_o = nc.dram_tensor("ssq", [2, N], FP32, kind="ExternalOutput")

    KT = DIN // P  # 8 contraction tiles

    with tile.TileContext(nc) as tc:
        with (
            tc.tile_pool(name="xw", bufs=1) as xw,
            tc.tile_pool(name="bigp", bufs=1) as bigp,
            tc.tile_pool(name="scr", bufs=2) as scr,
            tc.tile_pool(name="shp", bufs=2) as shp,
            tc.tile_pool(name="outq", bufs=2) as outq,
            tc.tile_pool(name="sqp", bufs=4) as sqp,
            tc.tile_pool(name="vst", bufs=1) as vst,
            tc.tile_pool(name="stgp", bufs=1) as stgp,
            tc.tile_pool(name="ps", bufs=4, space="PSUM") as ps,
        ):
            # ---- input DMAs: xt on sync queue, wt on scalar queue ----
            xt, wt = [], []
            for kt in range(KT):
                t = xw.tile([P, N], FP16, tag=f"xt{kt}")
                w = xw.tile([P, 3 * GF], FP16, tag=f"wt{kt}")
                if kt == 0:
                    nc.scalar.dma_start(w[:, 0:GF], wcat[0:P, 0:GF])
                    nc.sync.dma_start(t[:, 0:1024], xT[0:P, 0:1024])
                    nc.sync.dma_start(t[:, 1024:2048], xT[0:P, 1024:2048])
                    nc.scalar.dma_start(w[:, GF:3 * GF], wcat[0:P, GF:3 * GF])
                else:
                    nc.sync.dma_start(t[:], xT[kt * P:(kt + 1) * P, :])
                    nc.scalar.dma_start(w[:], wcat[kt * P:(kt + 1) * P, :])
                xt.append(t)
                wt.append(w)
            bias = xw.tile([P, 4], FP32, tag="bias")
            nc.scalar.dma_start(bias[:], bqk[:, :])
            winv = xw.tile([P, P], FP16, tag="winv")
            nc.scalar.dma_start(winv[:], invs[:, :])
            cosb = xw.tile([P, N], FP16, tag="cos")
            nc.scalar.dma_start(cosb[:], cosr[:, :])
            sinb = xw.tile([P, N], FP16, tag="sin")
            nc.scalar.dma_start(sinb[:], sinr[:, :])

            # v staging: 16 key-tiles x (4 heads x 65); ones columns set once
            vstage = vst.tile([P, 16 * 260], FP16, tag="vstage")
            ones = vst.tile([P, 64], FP16, tag="ones")
            nc.vector.memset(ones[:], 1.0)
            nc.vector.tensor_copy(
                vstage[:].rearrange("p (a h c) -> p a h c", a=16, h=4)[
                    :, :, :, 64:65],
                ones[:].rearrange("p (a h c) -> p a h c", a=16, h=4))

            sq = {}

            def qk_phase(t_idx, out_dram, streamed):
                # projection for one of q/k in transposed layout.
                col0 = t_idx * GF
                accs = [ps.tile([P, 1024], FP32, tag="acc",
                                name=f"acc{t_idx}_{i}") for i in range(4)]
                if streamed:
                    # kt-outer: compute behind the input DMA stream
                    for kt in range(KT):
                        for mt in range(2):
                            for nb in range(4):
                                nc.tensor.matmul(
                                    accs[mt * 2 + nb // 2][
                                        :, (nb % 2) * 512:(nb % 2 + 1) * 512],
                                    wt[kt][:, col0 + mt * P:col0 + (mt + 1) * P],
                                    xt[kt][:, nb * 512:(nb + 1) * 512],
                                    start=(kt == 0), stop=(kt == KT - 1),
                                )
                else:
                    for mt in range(2):
                        for nb in range(4):
                            for kt in range(KT):
                                nc.tensor.matmul(
                                    accs[mt * 2 + nb // 2][
                                        :, (nb % 2) * 512:(nb % 2 + 1) * 512],
                                    wt[kt][:, col0 + mt * P:col0 + (mt + 1) * P],
                                    xt[kt][:, nb * 512:(nb + 1) * 512],
                                    start=(kt == 0), stop=(kt == KT - 1),
                                )
                dmae = nc.scalar if t_idx == 0 else nc.sync
                for mt in range(2):
                    big = bigp.tile([P, N], FP16, tag=f"big{t_idx}_{mt}")
                    s = sqp.tile([P, N], FP16, tag=f"sq{t_idx}_{mt}")
                    for nbp in range(2):
                        nc.scalar.activation(
                            big[:, nbp * 1024:(nbp + 1) * 1024],
                            accs[mt * 2 + nbp][:], AF.Identity,
                            bias=bias[:, 2 * t_idx + mt:2 * t_idx + mt + 1])
                        # pre-rope squares for the weighted ssq, fused as
                        # (acc + bias)^2 on the ACT engine (idle here)
                        nc.scalar.activation(
                            s[:, nbp * 1024:(nbp + 1) * 1024],
                            accs[mt * 2 + nbp][:], AF.Square,
                            bias=bias[:, 2 * t_idx + mt:2 * t_idx + mt + 1])
                    sq[(t_idx, mt)] = s
                    # rope: rotate_half via 4 partition-block DMAs
                    sh = shp.tile([P, N], FP16, tag="sh")
                    for blk in range(4):
                        srcb = blk ^ 1
                        (dmae if blk < 2 else nc.gpsimd).dma_start(
                            sh[blk * 32:(blk + 1) * 32, :],
                            big[srcb * 32:(srcb + 1) * 32, :])
                    t2 = scr.tile([P, N], FP16, tag="t2")
                    nc.vector.tensor_mul(t2[:], big[:], cosb[:])
                    nc.vector.tensor_mul(sh[:], sh[:], sinb[:])
                    rr = outq.tile([P, N], FP16, tag="rr")
                    nc.vector.tensor_add(rr[:], t2[:], sh[:])
                    dmae.dma_start(out_dram[mt * P:(mt + 1) * P, :], rr[:])

            qk_phase(0, qR_o, streamed=True)
            qk_phase(1, kR_o, streamed=False)

            # ---- ssq: 32 identical output rows via all-equal lhsT columns ----
            stg = stgp.tile([1, 2 * N], FP32, tag="stg")
            for t_idx in range(2):
                for np2 in range(2):
                    sp = ps.tile([32, 1024], FP32, tag="acc",
                                 name=f"ssq{t_idx}_{np2}")
                    for nbi in range(2):
                        nb = np2 * 2 + nbi
                        for mt in range(2):
                            nc.tensor.matmul(
                                sp[:, nbi * 512:(nbi + 1) * 512],
                                winv[:, 32 * (2 * t_idx + mt):
                                     32 * (2 * t_idx + mt + 1)],
                                sq[(t_idx, mt)][:, nb * 512:(nb + 1) * 512],
                                start=(mt == 0), stop=(mt == 1),
                            )
                    nc.scalar.copy(
                        stg[0:1, t_idx * N + np2 * 1024:
                            t_idx * N + (np2 + 1) * 1024],
                        sp[0:1, :])
            for t_idx in range(2):
                nc.sync.dma_start(ssq_o[t_idx:t_idx + 1, :],
                                  stg[0:1, t_idx * N:(t_idx + 1) * N])

            # ---- v phase (tiles resident now) ----
            for tp in range(4):
                vacc = ps.tile([P, 1024], FP32, tag="acc", name=f"vacc{tp}")
                for ti in range(4):
                    tt = tp * 4 + ti
                    for kt in range(KT):
                        nc.tensor.matmul(
                            vacc[:, ti * 256:(ti + 1) * 256],
                            xt[kt][:, tt * P:(tt + 1) * P],
                            wt[kt][:, 2 * GF:3 * GF],
                            start=(kt == 0), stop=(kt == KT - 1),
                        )
                nc.scalar.copy(
                    vstage[:].rearrange("p (a h c) -> p a h c", a=16, h=4)[
                        :, tp * 4:(tp + 1) * 4, :, 0:64],
                    vacc[:].rearrange("p (a h c) -> p a h c", a=4, h=4))
                nc.scalar.dma_start(
                    v_o[:, tp * 1040:(tp + 1) * 1040],
                    vstage[:, tp * 1040:(tp + 1) * 1040])

    nc.compile()
    return nc


# ----------------------------------------------------------------- launch 2

def _build_l2():
    nc = bacc.Bacc("TRN2", target_bir_lowering=False, debug=False,
                   num_devices=NCORES)
    qR = nc.dram_tensor("qh", [GF, N], FP16, kind="ExternalInput")
    kR = nc.dram_tensor("kh", [GF, N], FP16, kind="ExternalInput")
    v_i = nc.dram_tensor("v65", [P, 16 * 260], FP16, kind="ExternalInput")
    wout_i = nc.dram_tensor("wout", [GF, DIN], FP16, kind="ExternalInput")
    part_o = nc.dram_tensor("part", [2, N, DIN], FP16, kind="ExternalOutput")

    IBW = 512        # query-block width
    NIB = N // IBW   # 4 query blocks
    NJT = N // P     # 16 key tiles

    with tile.TileContext(nc) as tc:
        with (
            tc.tile_pool(name="cst", bufs=1) as cst,
            tc.tile_pool(name="hatp", bufs=1) as hatp,
            tc.tile_pool(name="ptp", bufs=8) as ptp,
            tc.tile_pool(name="obig", bufs=1) as obigp,
            tc.tile_pool(name="onrm", bufs=2) as onrm,
            tc.tile_pool(name="outp", bufs=2) as outp,
            tc.tile_pool(name="tiny", bufs=4) as tiny,
            tc.tile_pool(name="psS", bufs=2, space="PSUM") as psS,
            tc.tile_pool(name="psA", bufs=1, space="PSUM") as psA,
            tc.tile_pool(name="psO", bufs=2, space="PSUM") as psO,
        ):
            # ---- loads: pair-0 tensors first so attention starts early ----
            khat, qhat = [], []
            for mt in range(2):
                kt_ = cst.tile([P, N], FP16, tag=f"kh{mt}")
                qt_ = cst.tile([P, N], FP16, tag=f"qh{mt}")
                khat.append(kt_)
                qhat.append(qt_)
            # first S step needs only kh0[:, :512] + qh0[:, :512]; lead with
            # small chunks split across both DMA queues, stream v in jt-chunks
            nc.sync.dma_start(khat[0][:, 0:512], kR[0:P, 0:512])
            nc.scalar.dma_start(qhat[0][:, 0:512], qR[0:P, 0:512])
            nc.sync.dma_start(khat[0][:, 512:N], kR[0:P, 512:N])
            nc.scalar.dma_start(qhat[0][:, 512:N], qR[0:P, 512:N])
            vbig = cst.tile([P, 16 * 260], FP16, tag="v")
            for vc in range(4):
                nc.sync.dma_start(vbig[:, vc * 1040:(vc + 1) * 1040],
                                  v_i[:, vc * 1040:(vc + 1) * 1040])
            wout = []
            for kt in range(2):
                w = cst.tile([P, DIN], FP16, tag=f"wo{kt}")
                nc.scalar.dma_start(w[:], wout_i[kt * P:(kt + 1) * P, :])
                wout.append(w)
            nc.scalar.dma_start(qhat[1][:], qR[P:2 * P, :])
            nc.sync.dma_start(khat[1][:], kR[P:2 * P, :])

            def vt_slice(jt, h):
                return vbig[:, jt * 260 + h * 65:jt * 260 + (h + 1) * 65]

            # ---- attention ----
            obig = [obigp.tile([P, N], FP16, tag=f"obig{pr}", name=f"ob{pr}")
                    for pr in range(2)]

            osb_pend = {}

            def emit_proj(pr, tt, eng, use_act=False):
                pss = [psO.tile([P, 512], FP32, tag="O",
                                name=f"pj{pr}_{tt}_{hf}")
                       for hf in range(2)]
                for half in range(2):
                    nc.tensor.matmul(
                        pss[half][:],
                        obig[pr][:, tt * P:(tt + 1) * P],
                        wout[pr][:, half * 512:(half + 1) * 512],
                        start=True, stop=True,
                    )
                if tt % 2 == 0:
                    osb_pend[pr] = outp.tile([P, 2 * DIN], FP16, tag="osb",
                                             name=f"osb{pr}_{tt}")
                osb = osb_pend[pr]
                base = (tt % 2) * DIN
                if use_act:
                    nc.scalar.copy(osb[:, base:base + 512], pss[0][:])
                    nc.vector.tensor_copy(osb[:, base + 512:base + 1024],
                                          pss[1][:])
                else:
                    eng.tensor_copy(osb[:, base:base + 512], pss[0][:])
                    eng.tensor_copy(osb[:, base + 512:base + 1024], pss[1][:])
                if tt % 2 == 1:
                    nc.sync.dma_start(
                        part_o[pr, (tt - 1) * P:(tt + 1) * P, :].rearrange(
                            "(a p) d -> p a d", p=P),
                        osb[:].rearrange("p (a d) -> p a d", a=2))

            def finish_block(pr, ib, o_ps):
                rinv, bc = [], []
                for sub in range(2):
                    d = tiny.tile([1, IBW], FP32, tag="dsb",
                                  name=f"d{pr}_{ib}_{sub}")
                    nc.vector.tensor_copy(d[:, :], o_ps[sub][64:65, :])
                    t = tiny.tile([1, IBW], FP32, tag="rinv",
                                  name=f"ri{pr}_{ib}_{sub}")
                    nc.vector.reciprocal_approx_fast(t[:, :], d[:, :])
                    rinv.append(t)
                for sub in range(2):
                    t = tiny.tile([64, IBW], FP32, tag="bc",
                                  name=f"bc{pr}_{ib}_{sub}")
                    nc.gpsimd.partition_broadcast(t[:, :], rinv[sub][:, :])
                    bc.append(t)
                nc.vector.tensor_mul(
                    obig[pr][0:64, ib * IBW:(ib + 1) * IBW],
                    o_ps[0][0:64, :], bc[0][:, :])
                onr = onrm.tile([64, IBW], FP16, tag="onr")
                nc.vector.tensor_mul(onr[:, :], o_ps[1][0:64, :], bc[1][:, :])
                nc.sync.dma_start(
                    obig[pr][64:128, ib * IBW:(ib + 1) * IBW], onr[:, :])

            steps = [(pr, ib, jt) for pr in range(2) for ib in range(NIB)
                     for jt in range(NJT)]
            p_sbs = {}
            o_ps_map = {}
            ready = []      # projection token-tiles whose obig cols are done
            launched = []

            def emit_s(step):
                pr, ib, jt = step
                s_ps = psS.tile([P, 2 * IBW], FP32, tag="S")
                for sub in range(2):
                    nc.tensor.matmul(
                        s_ps[:, sub * IBW:(sub + 1) * IBW],
                        khat[pr][sub * 64:(sub + 1) * 64, jt * P:(jt + 1) * P],
                        qhat[pr][sub * 64:(sub + 1) * 64,
                                 ib * IBW:(ib + 1) * IBW],
                        start=True, stop=True,
                        tile_position=(64 * sub, 0),
                    )
                p_sb = ptp.tile([P, 2 * IBW], FP16, tag="P",
                                name=f"p{pr}_{ib}_{jt}")
                nc.scalar.activation(p_sb[:, :], s_ps[:, :],
                                     AF.Exp, scale=0.125)
                p_sbs[step] = p_sb

            nproj = 0
            for k in range(6):
                emit_s(steps[k])
            for si, step in enumerate(steps):
                pr, ib, jt = step
                if si + 6 < len(steps):
                    emit_s(steps[si + 6])
                if (pr, ib) not in o_ps_map:
                    o_ps_map[(pr, ib)] = [
                        psA.tile([65, IBW], FP32, tag=f"oacc{s}",
                                 name=f"o{pr}_{ib}_{s}") for s in range(2)]
                o_ps = o_ps_map[(pr, ib)]
                p_sb = p_sbs.pop(step)
                for sub in range(2):
                    h = 2 * pr + sub
                    nc.tensor.matmul(
                        o_ps[sub][:, :],
                        vt_slice(jt, h),
                        p_sb[:, sub * IBW:(sub + 1) * IBW],
                        start=(jt == 0), stop=(jt == NJT - 1),
                    )
                if jt in (1, 4, 7, 10) and ready:
                    ptt = ready.pop(0)
                    emit_proj(ptt[0], ptt[1], nc.vector)
                    nproj += 1
                if jt == NJT - 1:
                    finish_block(pr, ib, o_ps)
                    ready.extend((pr, ib * 4 + i) for i in range(4))

            for ptt in ready:
                emit_proj(ptt[0], ptt[1], nc.vector, use_act=True)
                nproj += 1

    nc.compile()
    return nc


# ------------------------------------------------------------------- driver

def _rope_tables():
    half = DH // 2
    inv_freq = 1.0 / (ROPE_BASE ** (np.arange(half, dtype=np.float64) * 2.0
                                    / DH))
    freqs = np.arange(N, dtype=np.float64)[:, None] * inv_freq[None, :]
    cos = np.cos(freqs).T          # (32, N)
    sin = np.sin(freqs).T
    cos64 = np.concatenate([cos, cos], 0)            # (64, N)
    sin64 = np.concatenate([-sin, sin], 0)           # signed for rotate_half
    cos_t = np.ascontiguousarray(
        np.concatenate([cos64, cos64], 0).astype(np.float16))  # (128, N)
    sin_t = np.ascontiguousarray(
        np.concatenate([sin64, sin64], 0).astype(np.float16))
    return cos_t, sin_t


def kernel(input, w_qkv, b_qkv, q_scale, k_scale, w_out, b_out):
    trace = bool(os.environ.get("KERNEL_TRACE"))
    if "l1" not in _cache:
        _cache["l1"] = _build_l1()
    if "l2" not in _cache:
        _cache["l2"] = _build_l2()

    x = np.asarray(input, dtype=np.float32)
    w_qkv = np.asarray(w_qkv, dtype=np.float32)
    b_qkv = np.asarray(b_qkv, dtype=np.float32)
    qs = np.asarray(q_scale, dtype=np.float32)
    ks = np.asarray(k_scale, dtype=np.float32)
    w_out = np.asarray(w_out, dtype=np.float32)
    b_out = np.asarray(b_out, dtype=np.float32)

    wq = w_qkv[:, :DQ] * qs[None, :]
    wk = w_qkv[:, DQ:2 * DQ] * ks[None, :]
    wv = w_qkv[:, 2 * DQ:]
    bq = b_qkv[:DQ] * qs
    bk = b_qkv[DQ:2 * DQ] * ks
    bv = b_qkv[2 * DQ:]

    xT = [np.ascontiguousarray(x[b].T.astype(np.float16)) for b in range(B)]
    cos_t, sin_t = _rope_tables()

    def col4(vec256_a, vec256_b):
        # -> (128, 4): [a_mt0 | a_mt1 | b_mt0 | b_mt1]
        return np.ascontiguousarray(np.stack(
            [vec256_a[:P], vec256_a[P:], vec256_b[:P], vec256_b[P:]],
            axis=1).astype(np.float32))

    in1 = []
    for c in range(NCORES):
        b, g = divmod(c, NGROUP)
        sl = slice(g * GF, (g + 1) * GF)
        wcat = np.ascontiguousarray(np.concatenate(
            [wq[:, sl], wk[:, sl], wv[:, sl]], axis=1).astype(np.float16))
        in1.append({
            "xT": xT[b],
            "wcat": wcat,
            "bqk": col4(bq[sl], bk[sl]),
            "invs": np.ascontiguousarray(np.repeat(
                col4(1.0 / np.square(qs[sl]), 1.0 / np.square(ks[sl])),
                32, axis=1).astype(np.float16)),
            "cosr": cos_t,
            "sinr": sin_t,
        })

    r1 = run_bass_kernel_spmd(_cache["l1"], in1,
                              core_ids=list(range(NCORES)), trace=trace)
    if trace:
        LAST_EXEC_NS["l1"] = r1.exec_time_ns
        LAST_RESULTS["l1"] = r1

    # host: combine partial ssq -> rsqrt factors
    tabs = {}
    for b in range(B):
        sq_q = np.zeros(N, np.float64)
        sq_k = np.zeros(N, np.float64)
        for g in range(NGROUP):
            ssq = r1.results[NGROUP * b + g]["ssq"].astype(np.float64)
            sq_q += ssq[0]
            sq_k += ssq[1]
        tabs[b] = {
            "r_q": (1.0 / np.sqrt(sq_q / DQ + EPS)).astype(np.float32),
            "r_k": (1.0 / np.sqrt(sq_k / DQ + EPS)).astype(np.float32),
        }

    in2 = []
    for c in range(NCORES):
        b, g = divmod(c, NGROUP)
        sl = slice(g * GF, (g + 1) * GF)
        in2.append({
            "qh": np.ascontiguousarray(
                r1.results[c]["qR"].astype(np.float32)
                * tabs[b]["r_q"][None, :]).astype(np.float16),
            "kh": np.ascontiguousarray(
                r1.results[c]["kR"].astype(np.float32)
                * tabs[b]["r_k"][None, :]).astype(np.float16),
            "v65": r1.results[c]["v65"],
            "wout": np.ascontiguousarray(w_out[sl, :].astype(np.float16)),
        })

    r2 = run_bass_kernel_spmd(_cache["l2"], in2,
                              core_ids=list(range(NCORES)), trace=trace)
    if trace:
        LAST_EXEC_NS["l2"] = r2.exec_time_ns
        LAST_RESULTS["l2"] = r2

    base = (bv.astype(np.float64) @ w_out.astype(np.float64)
            + b_out.astype(np.float64))
    out = np.zeros((B, N, DIN), np.float32)
    for b in range(B):
        acc = np.zeros((N, DIN), np.float64)
        for g in range(NGROUP):
            p = r2.results[NGROUP * b + g]["part"].astype(np.float64)
            acc += p[0]
            acc += p[1]
        out[b] = (acc + base[None, :]).astype(np.float32)
    return out
